# revision 2
# baseline (speedup 1.0000x reference)
"""Trainium2 Bass kernel for nn_DCAA_57604101374115 (moe_routing).

Per-sample pipelined implementation. Each of the 8 cores gets 2 samples
(pure data parallel over batch 16). Per core, the two samples run as
pipeline units so sample 1's HBM load hides under sample 0's compute and
sample 0's store hides sample 1's compute (the DMA bus is a single serial
360 GB/s resource in the cost model; per-core traffic is 6.4 MB in +
12.8 MB out).

Per-sample layout: SBUF partitions = (image half in {top,bottom}) x
(channel 0..63); free dim = (row-in-half 0..55, col 0..111). All per-op
costs (free-size based) match the packed 2-sample layout, and the 3x3
depthwise halo at the half boundary is produced by two extra 112-col
"swap block" matmuls (weight maps half h input to half 1-h output).

Engine split per sample:
  PE   : dynamic 1x1 conv (14x448-col block-diag matmuls + 2 halo) and
         34 rows of the depthwise conv as diag-matmul PSUM accumulation.
  DVE  : 12 rows of depthwise (scalar_tensor_tensor MACs), routing/SE
         small ops, gating (tensor_scalar at the 2x SBUF rate).
  Pool : 10 rows of depthwise, memsets, some evac/gating.
  ACT  : all BN+ReLU PSUM/SBUF evacuations with accum_out providing the
         spatial sums for routing/SE, sigmoids, some gating.
PE p-state: dummy matmuls keep the tensor engine's busy-run alive across
phase gaps so real matmuls run at the full 2.4 GHz rate.
"""

import numpy as np
from contextlib import ExitStack

import concourse.bass as bass
import concourse.tile as tile
from concourse import bacc, mybir
from concourse.bass_utils import run_bass_kernel_spmd

# ---------------- problem constants ----------------
B, C_IN, H, W = 16, 64, 112, 112
INIT = 64
NEW = 64
E = 4
SE_HID = 32
EPS = 1e-5
NCORES = 8
BLOC = B // NCORES          # 2 samples per core
P = 128
HH = H // 2                 # 56 rows per half
IMG = HH * W                # 6272 elements per partition per sample
PADH, PADW = HH + 2, W + 2  # 58 x 114 padded x1
HWF = float(H * W)

f32 = mybir.dt.float32
f32r = mybir.dt.float32r
bf16 = mybir.dt.bfloat16
AX = mybir.AxisListType.X
MULT = mybir.AluOpType.mult
ADD = mybir.AluOpType.add
MAX = mybir.AluOpType.max
RELU = mybir.ActivationFunctionType.Relu
SIGM = mybir.ActivationFunctionType.Sigmoid
COPY = mybir.ActivationFunctionType.Copy

# ---------------- tuning knobs ----------------
NLOAD = 4                   # load pieces per sample (14 rows each)
DUM_A = 0                  # PE dummies during load0 (256 cols ~= 107-213 ns each)
DUM_B = 0                   # bridge r1_0 -> M1_0
DUM_C = 0                  # bridge M1_0 -> r2_0
DUM_D = 0                   # bridge r2_0 -> DW_0
DUM_SE0 = 0                # bridge DW_0 G3 -> se_0 matmuls
DUM_E = 0                   # bridge M1_1 -> r2_1
DUM_F = 0
DUM_SE1 = 0
DUMCOL = 256                # dummy matmul width (>=256 to stay 1 cyc/row)

# depthwise row split per sample: PE chunk list (rows each), DVE/Pool 2-row chunks
PE_DW = [4, 4, 4, 4, 4, 4, 4, 4, 4, 4, 3, 3]  # 46 rows
PE_DW_GROUPS = [(0, 4), (4, 3), (7, 3), (10, 2)]
DVE_DW_ROWS = [(46, 4), (50, 3), (53, 3)]  # (row0, nrows)

# M1 evacuation engine per chunk (14 chunks x 4 rows): A=ACT D=DVE P=Pool
M1_EVAC_0 = ['A', 'A', 'D', 'A', 'D', 'D', 'A', 'D', 'A', 'A', 'D', 'D', 'A', 'A']
M1_EVAC_1 = ['A', 'A', 'A', 'A', 'D', 'D', 'D', 'A', 'A', 'D', 'D', 'D', 'A', 'A']

# gate piece engine per (x1a, x1b, x2a, x2b)
GATE_ENG = ['D', 'A', 'D', 'P']

# ---------------- const blob ----------------
# column layout: [early | mid | se]
_EARLY = {"w1T": E * 64, "rw1": E, "rb1": 1, "maskE": E, "bn1b": 1}
_MID = {"i128": P, "rw2": E, "rb2": 1, "w2": E * 9, "bn2b": 1}
_SE = {"sew1a": SE_HID, "sew1b": SE_HID, "seb1": 1,
       "sew2a": P, "sew2b": P, "seb2a": 1, "seb2b": 1}
_OFF = {}
_off = 0
for _sec in (_EARLY, _MID, _SE):
    for _n, _w in _sec.items():
        _OFF[_n] = _off
        _off += _w
CBW = _off
EARLY_W = sum(_EARLY.values())
MID_W = sum(_MID.values())
SE_W = sum(_SE.values())


def _pack_consts(inp):
    n = {k: np.asarray(v, dtype=np.float32) for k, v in inp.items()}
    s1 = n["bn1_g"] / np.sqrt(n["bn1_v"] + EPS)
    s2 = n["bn2_g"] / np.sqrt(n["bn2_v"] + EPS)
    w1m = n["w1"][:, :, :, 0, 0] * s1[None, :, None]      # [E, O, I], bn1 scale folded
    w2m = n["w2"][:, :, 0].reshape(E, NEW, 9) * s2[None, :, None]

    c = {}
    # w1T[(h,ci), (e,co)] = w1m[e, co, ci]
    c["w1T"] = np.tile(w1m.transpose(2, 0, 1).reshape(C_IN, E * 64), (2, 1))
    c["rw1"] = np.tile(n["rw1"].T / HWF, (2, 1))          # [P, E]
    c["rb1"] = n["rb1"][:, None]                          # [E, 1]
    c["maskE"] = np.eye(E, dtype=np.float32)
    c["bn1b"] = np.tile(n["bn1_b"] - n["bn1_m"] * s1, 2)[:, None]
    c["i128"] = np.eye(P, dtype=np.float32)
    c["rw2"] = np.tile(n["rw2"].T / HWF, (2, 1))
    c["rb2"] = n["rb2"][:, None]
    c["w2"] = np.tile(w2m.transpose(1, 0, 2).reshape(NEW, E * 9), (2, 1))
    c["bn2b"] = np.tile(n["bn2_b"] - n["bn2_m"] * s2, 2)[:, None]
    c["sew1a"] = np.tile(n["se_w1"][:, :64].T / HWF, (2, 1))
    c["sew1b"] = np.tile(n["se_w1"][:, 64:].T / HWF, (2, 1))
    c["seb1"] = n["se_b1"][:, None]
    c["sew2a"] = np.zeros((SE_HID, P), np.float32)
    c["sew2a"][:, :64] = n["se_w2"][:64].T
    c["sew2a"][:, 64:] = n["se_w2"][:64].T
    c["sew2b"] = np.zeros((SE_HID, P), np.float32)
    c["sew2b"][:, :64] = n["se_w2"][64:].T
    c["sew2b"][:, 64:] = n["se_w2"][64:].T
    c["seb2a"] = np.tile(n["se_b2"][:64], 2)[:, None]
    c["seb2b"] = np.tile(n["se_b2"][64:], 2)[:, None]

    blob = np.zeros((P, CBW), np.float32)
    for sec in (_EARLY, _MID, _SE):
        for name, w in sec.items():
            v = c[name]
            blob[:v.shape[0], _OFF[name]:_OFF[name] + w] = v
    return blob


# ---------------- device kernel ----------------
def _emit(tc, x_d, y_d, cblob_d):
    nc = tc.nc
    with ExitStack() as ctx:
        const = ctx.enter_context(tc.tile_pool(name="const", bufs=1))
        data = ctx.enter_context(tc.tile_pool(name="data", bufs=1))
        small = ctx.enter_context(tc.tile_pool(name="small", bufs=1))
        stage = ctx.enter_context(tc.tile_pool(name="stage", bufs=1))
        psum = ctx.enter_context(tc.tile_pool(name="psum", bufs=1, space="PSUM"))

        cblob = const.tile([P, CBW], f32)
        ct = {}
        for sec in (_EARLY, _MID, _SE):
            for name, w in sec.items():
                rows = {"rb1": E, "maskE": E, "rb2": E, "seb1": SE_HID,
                        "sew2a": SE_HID, "sew2b": SE_HID}.get(name, P)
                ct[name] = cblob[0:rows, _OFF[name]:_OFF[name] + w]

        # ---- DMA stream (SP): consts + loads; stores are emitted later ----
        nc.sync.dma_start(cblob[:, 0:EARLY_W], cblob_d.ap()[:, 0:EARLY_W])
        x_ap = x_d.ap().rearrange("b c r w -> b c (r w)")
        xb, xr, x1pad = [], [], []
        for s in range(BLOC):
            xb.append(data.tile([P, IMG], f32, name=f"xb{s}"))
            xr.append(data.tile([P, IMG], f32r, name=f"xr{s}"))
            x1pad.append(data.tile([P, PADH * PADW], f32r, name=f"x1pad{s}"))
        LP = IMG // NLOAD

        def load_piece(s, i):
            for hf in range(2):
                nc.sync.dma_start(
                    xb[s][64 * hf:64 * hf + 64, LP * i:LP * (i + 1)],
                    x_ap[s, :, HH * W * hf + LP * i:HH * W * hf + LP * (i + 1)])

        for i in range(NLOAD):
            load_piece(0, i)
        nc.sync.dma_start(cblob[:, EARLY_W:EARLY_W + MID_W],
                          cblob_d.ap()[:, EARLY_W:EARLY_W + MID_W])
        for i in range(NLOAD):
            load_piece(1, i)
        nc.sync.dma_start(cblob[:, EARLY_W + MID_W:CBW],
                          cblob_d.ap()[:, EARLY_W + MID_W:CBW])

        xrv = [xr[s].rearrange("p (r w) -> p r w", w=W) for s in range(BLOC)]
        x1v = [x1pad[s].rearrange("p (r w) -> p r w", w=PADW) for s in range(BLOC)]
        # depthwise outputs reuse the landing buffers (dead after the casts)
        x2v = [xb[0].rearrange("p (r w) -> p r w", w=W),
               xb[1].rearrange("p (r w) -> p r w", w=W)]

        # ---- small tiles ----
        def sm(shape, nm, dt=f32):
            return small.tile(shape, dt, name=nm)
        xsum = [sm([P, NLOAD], f"xsum{s}") for s in range(2)]
        x1s = [sm([P, 16], f"x1s{s}") for s in range(2)]
        x2s = [sm([P, 24], f"x2s{s}") for s in range(2)]
        xsumT = [sm([P, 1], f"xsumT{s}") for s in range(2)]
        x1sT = [sm([P, 1], f"x1sT{s}") for s in range(2)]
        x2sT = [sm([P, 1], f"x2sT{s}") for s in range(2)]
        r1s = [sm([E, 1], f"r1s{s}") for s in range(2)]
        r2s = [sm([E, 1], f"r2s{s}") for s in range(2)]
        r1m = [sm([E, E], f"r1m{s}") for s in range(2)]
        r2m = [sm([E, E], f"r2m{s}") for s in range(2)]
        r1b = [sm([P, E], f"r1b{s}") for s in range(2)]
        r2b = [sm([P, E], f"r2b{s}") for s in range(2)]
        k1c = [sm([P, 64], f"k1c{s}") for s in range(2)]
        k2c = [sm([P, 9], f"k2c{s}") for s in range(2)]
        mm1w = [sm([P, P], f"mm1w{s}", f32r) for s in range(2)]
        mm1sw = [sm([P, P], f"mm1sw{s}", f32r) for s in range(2)]
        dwt = [sm([P, 9 * P], f"dwt{s}", f32r) for s in range(2)]
        seh = [sm([SE_HID, 1], f"seh{s}") for s in range(2)]
        s1c = [sm([P, 1], f"s1c{s}") for s in range(2)]
        s2c = [sm([P, 1], f"s2c{s}") for s in range(2)]
        ones4 = sm([E, P], "ones4")
        warm = sm([1, 1], "warm")

        # ---- ACT table warm + structural zeroing (all idle time).
        # f32r tiles cannot be memset; zero them with x0.0 compute writes
        # from an already-loaded const region (waits the early-const DMA).
        nc.scalar.activation(warm[:], cblob[0:1, 0:1], SIGM, bias=0.0, scale=1.0)
        nc.gpsimd.memset(ones4[:], 1.0)
        zsrc = cblob[:, 0:PADW]
        for s in range(BLOC):
            nc.vector.tensor_scalar_mul(mm1w[s][:], cblob[:, 0:P], 0.0)
            nc.vector.tensor_scalar_mul(mm1sw[s][:], cblob[:, 0:P], 0.0)
            nc.vector.tensor_scalar_mul(x1v[s][:, 0, :], zsrc[:, 0:PADW], 0.0)
            nc.vector.tensor_scalar_mul(x1v[s][:, PADH - 1, :], zsrc[:, 0:PADW], 0.0)
            nc.vector.tensor_scalar_mul(x1v[s][:, :, 0], zsrc[:, 0:PADH], 0.0)
            nc.vector.tensor_scalar_mul(x1v[s][:, :, PADW - 1], zsrc[:, 0:PADH], 0.0)

        # ---- helpers ----
        dum_ps = psum.tile([P, DUMCOL], f32, tag="dum", bufs=1)
        dum_src = small.tile([P, DUMCOL], f32r, name="dum_src")
        dgate = small.tile([P, DUMCOL], f32r, name="dgate")
        nc.vector.tensor_scalar_mul(dum_src[:], cblob[:, 0:DUMCOL], 0.0)
        nc.vector.tensor_scalar_mul(dgate[:], cblob[:, 0:DUMCOL], 0.0)
        dum_lhs = dum_src[:, 0:P]

        def dummies(n):
            # ungated: ready as soon as dum_src exists
            for _ in range(n):
                nc.tensor.matmul(dum_ps[:], dum_lhs, dum_src[:, 0:DUMCOL],
                                 start=True, stop=True)

        def dgate_mark():
            # rewrite the gate tile on DVE at this point in its stream; the
            # next gated-dummy batch becomes ready only once this runs
            nc.vector.tensor_scalar_mul(dgate[:], cblob[:, 0:DUMCOL], 0.0)

        def gdummies(n):
            # gated: wait the latest dgate version, so the scheduler cannot
            # hoist these bridge dummies ahead of their phase
            for _ in range(n):
                nc.tensor.matmul(dum_ps[:], dum_lhs, dgate[:, 0:DUMCOL],
                                 start=True, stop=True)

        def xsum_partial(s, i, eng):
            # fp32 -> fp32r rounding cast + spatial-sum side channel
            src = xb[s][:, LP * i:LP * (i + 1)]
            dst = xr[s][:, LP * i:LP * (i + 1)]
            if eng == 'A':
                nc.scalar.activation(dst, src, COPY, bias=0.0, scale=1.0,
                                     accum_out=xsum[s][:, i:i + 1])
            else:
                nc.vector.tensor_scalar(out=dst, in0=src, scalar1=1.0,
                                        scalar2=None, op0=MULT, op1=ADD,
                                        accum_out=xsum[s][:, i:i + 1])

        def r1_mm_pre(s):
            ps = psum.tile([E, 1], f32, tag="sm", bufs=1, name="smps")
            nc.tensor.matmul(ps[:], ct["rw1"], xsumT[s][:], start=True, stop=True)
            return ps

        def r_chain_smalls(s, pre_ps, rs, rm, rb_, kc, wsrc, ncols, rbias):
            # ACT sigmoid; DVE mask-mul; PE bcast matmul; DVE copy + mix
            nc.scalar.activation(rs[:], pre_ps[:], SIGM, bias=rbias, scale=1.0)
            nc.vector.tensor_scalar_mul(rm[:], ct["maskE"], rs[:, 0:1])
            bp = psum.tile([P, E], f32, tag="sm", bufs=1, name="smps")
            nc.tensor.matmul(bp[:], ones4[:], rm[:], start=True, stop=True)
            nc.vector.tensor_copy(rb_[:], bp[:])
            nc.vector.tensor_scalar_mul(kc[:], wsrc[:, 0:ncols], rb_[:, 0:1])
            for e in range(1, E):
                nc.vector.scalar_tensor_tensor(
                    kc[:], wsrc[:, e * ncols:(e + 1) * ncols],
                    rb_[:, e:e + 1], kc[:], op0=MULT, op1=ADD)

        def build_mm1(s):
            nc.vector.tensor_scalar_mul(mm1w[s][0:64, 0:64], k1c[s][0:64, :], 1.0)
            nc.vector.tensor_scalar_mul(mm1w[s][64:128, 64:128], k1c[s][64:128, :], 1.0)
            nc.vector.tensor_scalar_mul(mm1sw[s][0:64, 64:128], k1c[s][0:64, :], 1.0)
            nc.vector.tensor_scalar_mul(mm1sw[s][64:128, 0:64], k1c[s][64:128, :], 1.0)

        def m1_chunk(s, g):
            ps = psum.tile([P, 448], f32, tag="mm", bufs=6, name="mmps")
            nc.tensor.matmul(ps[:], mm1w[s][:],
                             xrv[s][:, 4 * g:4 * g + 4, :],
                             start=True, stop=True)
            return ps

        def m1_evac(s, g, ps, eng):
            dst = x1v[s][:, 1 + 4 * g:1 + 4 * g + 4, 1:1 + W]
            src = ps[:, 0:448].rearrange("p (r w) -> p r w", w=W)
            acc = x1s[s][:, g:g + 1]
            if eng == 'A':
                nc.scalar.activation(dst, src, RELU, bias=ct["bn1b"], scale=1.0,
                                     accum_out=acc)
            else:
                # two ops: bias+relu, then in-place copy carrying accum_out
                # (DVE two-scalar ts with accum_out mis-executes on HW)
                nc.vector.tensor_scalar(out=dst, in0=src, scalar1=ct["bn1b"],
                                        scalar2=0.0, op0=ADD, op1=MAX)
                nc.vector.tensor_scalar(out=dst, in0=dst.bitcast(f32),
                                        scalar1=1.0, scalar2=None,
                                        op0=MULT, op1=ADD, accum_out=acc)

        def m1_halo(s):
            # swap-block conv of half-boundary rows -> pad halo rows
            h0 = psum.tile([P, 112], f32, tag="mm", bufs=6, name="mmps")
            nc.tensor.matmul(h0[:], mm1sw[s][:], xrv[s][:, 0, :],
                             start=True, stop=True)
            nc.scalar.activation(x1v[s][0:64, PADH - 1, 1:1 + W], h0[0:64, :],
                                 RELU, bias=ct["bn1b"][0:64, :], scale=1.0)
            h1 = psum.tile([P, 112], f32, tag="mm", bufs=6, name="mmps")
            nc.tensor.matmul(h1[:], mm1sw[s][:], xrv[s][:, HH - 1, :],
                             start=True, stop=True)
            nc.scalar.activation(x1v[s][64:128, 0, 1:1 + W], h1[64:128, :],
                                 RELU, bias=ct["bn1b"][64:128, :], scale=1.0)

        def build_dwt(s):
            for t in range(9):
                nc.vector.tensor_scalar_mul(dwt[s][:, t * P:(t + 1) * P],
                                            ct["i128"], k2c[s][:, t:t + 1])

        def dw_pe_group(s, gi):
            c0, nch = PE_DW_GROUPS[gi]
            pss = []
            rows = []
            for ci in range(nch):
                nr = PE_DW[c0 + ci]
                r0 = sum(PE_DW[:c0 + ci])
                pss.append(psum.tile([P, 448], f32, tag="mm", bufs=6, name="mmps"))
                rows.append((r0, nr))
            for t in range(9):
                dy, dx = divmod(t, 3)
                for ci in range(nch):
                    r0, nr = rows[ci]
                    nc.tensor.matmul(
                        pss[ci][:, 0:nr * W],
                        dwt[s][:, t * P:(t + 1) * P],
                        x1v[s][:, r0 + dy:r0 + dy + nr, dx:dx + W],
                        start=(t == 0), stop=(t == 8))
            return pss, rows

        def dw_pe_evac(s, pss, rows, cols):
            for ci, (ps, (r0, nr)) in enumerate(zip(pss, rows)):
                nc.scalar.activation(
                    x2v[s][:, r0:r0 + nr, :],
                    ps[:, 0:nr * W].rearrange("p (r w) -> p r w", w=W),
                    RELU, bias=ct["bn2b"], scale=1.0,
                    accum_out=x2s[s][:, cols + ci:cols + ci + 1])

        def dw_vec_chunk(s, r0, nr, eng):
            acc = stage.tile([P, 448], f32, tag="acc" + eng, bufs=3, name="acc")
            accv = acc[:, 0:nr * W].rearrange("p (r w) -> p r w", w=W)
            e = nc.vector if eng == 'D' else nc.gpsimd
            e.tensor_scalar(out=accv, in0=x1v[s][:, r0:r0 + nr, 0:W].bitcast(f32),
                            scalar1=k2c[s][:, 0:1], scalar2=ct["bn2b"],
                            op0=MULT, op1=ADD)
            for t in range(1, 9):
                dy, dx = divmod(t, 3)
                e.scalar_tensor_tensor(
                    accv, x1v[s][:, r0 + dy:r0 + dy + nr, dx:dx + W].bitcast(f32),
                    k2c[s][:, t:t + 1], accv, op0=MULT, op1=ADD)
            return acc

        def dw_pool_chunk(s, r0, nr):
            # Pool lacks scalar_tensor_tensor: tap product via tensor_scalar
            # into a temp, accumulate via tensor_tensor add.
            acc = stage.tile([P, 448], f32, tag="accP", bufs=2, name="acc")
            tmp = stage.tile([P, 448], f32, tag="tmpP", bufs=2, name="tmp")
            accv = acc[:, 0:nr * W].rearrange("p (r w) -> p r w", w=W)
            tmpv = tmp[:, 0:nr * W].rearrange("p (r w) -> p r w", w=W)
            nc.gpsimd.tensor_scalar(out=accv,
                                    in0=x1v[s][:, r0:r0 + nr, 0:W].bitcast(f32),
                                    scalar1=k2c[s][:, 0:1], scalar2=ct["bn2b"],
                                    op0=MULT, op1=ADD)
            for t in range(1, 9):
                dy, dx = divmod(t, 3)
                nc.gpsimd.tensor_scalar_mul(
                    tmpv, x1v[s][:, r0 + dy:r0 + dy + nr, dx:dx + W].bitcast(f32),
                    k2c[s][:, t:t + 1])
                nc.gpsimd.tensor_tensor(out=accv, in0=accv, in1=tmpv, op=ADD)
            return acc

        def dw_vec_evac(s, r0, nr, acc, col, eng):
            # relu into acc (bias folded into tap 0), then copy acc -> x2
            # carrying the accum_out side-channel (baseline-proven shapes)
            e = nc.vector if eng == 'D' else nc.gpsimd
            accv = acc[:, 0:nr * W].rearrange("p (r w) -> p r w", w=W)
            e.tensor_scalar(out=accv, in0=accv, scalar1=0.0, scalar2=0.0,
                            op0=MAX, op1=ADD)
            e.tensor_scalar(out=x2v[s][:, r0:r0 + nr, :], in0=accv,
                            scalar1=1.0, scalar2=None, op0=MULT, op1=ADD,
                            accum_out=x2s[s][:, col:col + 1])

        def se_mms(s):
            se1 = psum.tile([SE_HID, 1], f32, tag="sm", bufs=1, name="smps")
            nc.tensor.matmul(se1[:], ct["sew1a"], x1sT[s][:], start=True, stop=False)
            nc.tensor.matmul(se1[:], ct["sew1b"], x2sT[s][:], start=False, stop=True)
            nc.scalar.activation(seh[s][:], se1[:], RELU, bias=ct["seb1"], scale=1.0)
            g1 = psum.tile([P, 1], f32, tag="sm", bufs=1, name="smps")
            nc.tensor.matmul(g1[:], ct["sew2a"], seh[s][:], start=True, stop=True)
            nc.scalar.activation(s1c[s][:], g1[:], SIGM, bias=ct["seb2a"], scale=1.0)
            g2 = psum.tile([P, 1], f32, tag="sm", bufs=1, name="smps")
            nc.tensor.matmul(g2[:], ct["sew2b"], seh[s][:], start=True, stop=True)
            nc.scalar.activation(s2c[s][:], g2[:], SIGM, bias=ct["seb2b"], scale=1.0)

        GR = 14                  # gate/store piece rows

        def gate_piece(s, half_src, pi, eng):
            # half_src: 0 -> x1 (from x1pad interior), 1 -> x2
            r0 = GR * pi
            if half_src == 0:
                src = x1v[s][:, 1 + r0:1 + r0 + GR, 1:1 + W].bitcast(f32)
                sc = s1c[s][:, 0:1]
            else:
                src = x2v[s][:, r0:r0 + GR, :]
                sc = s2c[s][:, 0:1]
            nbuf = {'D': 2, 'A': 1, 'P': 2}[eng]
            gst = stage.tile([P, GR * W], f32, tag="gst" + eng, bufs=nbuf,
                             name="gst")
            gv = gst.rearrange("p (r w) -> p r w", w=W)
            if eng == 'D':
                nc.vector.tensor_scalar_mul(gv, src, sc)
            elif eng == 'P':
                nc.gpsimd.tensor_scalar_mul(gv, src, sc)
            else:
                nc.scalar.activation(gv, src, COPY, bias=0.0, scale=sc)
            return gst

        y_ap1 = y_d.ap()[:, 0:64].rearrange("b c r w -> b c (r w)")
        y_ap2 = y_d.ap()[:, 64:128].rearrange("b c r w -> b c (r w)")

        def store_piece(s, half_src, pi, gst):
            dst = (y_ap1 if half_src == 0 else y_ap2)
            for hf in range(2):
                off = HH * W * hf + GR * W * pi
                nc.sync.dma_start(dst[s, :, off:off + GR * W],
                                  gst[64 * hf:64 * hf + 64, :])

        # ================= emission schedule =================
        # -- load0 window --
        dummies(DUM_A)
        xsum_partial(0, 0, 'A')
        xsum_partial(0, 1, 'A')
        xsum_partial(0, 2, 'A')
        xsum_partial(0, 3, 'D')
        nc.vector.reduce_sum(out=xsumT[0][:], in_=xsum[0][:, 0:NLOAD], axis=AX)
        pre0 = r1_mm_pre(0)
        r_chain_smalls(0, pre0, r1s[0], r1m[0], r1b[0], k1c[0], ct["w1T"], 64,
                       ct["rb1"])
        build_mm1(0)
        dgate_mark()
        gdummies(DUM_B)

        # -- M1_0 --
        for g in range(14):
            ps = m1_chunk(0, g)
            m1_evac(0, g, ps, M1_EVAC_0[g])
        m1_halo(0)
        nc.vector.reduce_sum(out=x1sT[0][:], in_=x1s[0][:, 0:14], axis=AX)
        dgate_mark()
        gdummies(DUM_C)

        # -- r2_0 --
        pre = psum.tile([E, 1], f32, tag="sm", bufs=1, name="smps")
        nc.tensor.matmul(pre[:], ct["rw2"], x1sT[0][:], start=True, stop=True)
        r_chain_smalls(0, pre, r2s[0], r2m[0], r2b[0], k2c[0], ct["w2"], 9,
                       ct["rb2"])
        build_dwt(0)
        dgate_mark()
        gdummies(DUM_D)

        # -- DW_0 + sample-1 prep interleave --
        xsum_partial(1, 0, 'A')
        xsum_partial(1, 1, 'A')
        xsum_partial(1, 2, 'D')
        xsum_partial(1, 3, 'D')
        nc.vector.reduce_sum(out=xsumT[1][:], in_=xsum[1][:, 0:NLOAD], axis=AX)
        pss1, rows1 = dw_pe_group(0, 0)  # PE G1 (4 chunks)
        dw_pe_evac(0, pss1, rows1, 0)

        # r1_1 chain (PE mms slot in mid-DW stream)
        pre1 = r1_mm_pre(1)
        r_chain_smalls(1, pre1, r1s[1], r1m[1], r1b[1], k1c[1], ct["w1T"], 64,
                       ct["rb1"])
        build_mm1(1)

        # dve DW chunks 0-1
        r0, nr = DVE_DW_ROWS[0]
        acc = dw_vec_chunk(0, r0, nr, 'D')
        dw_vec_evac(0, r0, nr, acc, 11, 'D')

        pss2, rows2 = dw_pe_group(0, 1)  # PE G2 (3 chunks)
        dw_pe_evac(0, pss2, rows2, 4)
        r0, nr = DVE_DW_ROWS[1]
        acc = dw_vec_chunk(0, r0, nr, 'D')
        dw_vec_evac(0, r0, nr, acc, 12, 'D')
        pss3, rows3 = dw_pe_group(0, 2)  # PE G3
        dw_pe_evac(0, pss3, rows3, 7)
        r0, nr = DVE_DW_ROWS[2]
        acc = dw_vec_chunk(0, r0, nr, 'D')
        dw_vec_evac(0, r0, nr, acc, 13, 'D')
        pss4, rows4 = dw_pe_group(0, 3)  # PE G4
        dw_pe_evac(0, pss4, rows4, 14)

        nc.vector.reduce_sum(out=x2sT[0][:], in_=x2s[0][:, 0:16], axis=AX)
        dgate_mark()
        gdummies(DUM_SE0)
        se_mms(0)

        # -- gate_0 pieces + M1_1 --
        GATE_SEQ = [(0, 0, 'D'), (0, 1, 'P'), (0, 2, 'A'), (0, 3, 'P'),
                    (1, 0, 'D'), (1, 1, 'P'), (1, 2, 'A'), (1, 3, 'P')]
        gp = gate_piece(0, *GATE_SEQ[0])
        store_piece(0, GATE_SEQ[0][0], GATE_SEQ[0][1], gp)
        gp = gate_piece(0, *GATE_SEQ[1])
        store_piece(0, GATE_SEQ[1][0], GATE_SEQ[1][1], gp)
        for g in range(14):
            ps = m1_chunk(1, g)
            m1_evac(1, g, ps, M1_EVAC_1[g])
        m1_halo(1)
        for hs, pi, eng in GATE_SEQ[2:]:
            gp = gate_piece(0, hs, pi, eng)
            store_piece(0, hs, pi, gp)
        nc.vector.reduce_sum(out=x1sT[1][:], in_=x1s[1][:, 0:14], axis=AX)
        dgate_mark()
        gdummies(DUM_E)

        # -- r2_1 --
        pre = psum.tile([E, 1], f32, tag="sm", bufs=1, name="smps")
        nc.tensor.matmul(pre[:], ct["rw2"], x1sT[1][:], start=True, stop=True)
        r_chain_smalls(1, pre, r2s[1], r2m[1], r2b[1], k2c[1], ct["w2"], 9,
                       ct["rb2"])
        build_dwt(1)
        dgate_mark()
        gdummies(DUM_F)

        # -- DW_1 --
        pss1, rows1 = dw_pe_group(1, 0)
        dw_pe_evac(1, pss1, rows1, 0)
        r0, nr = DVE_DW_ROWS[0]
        acc = dw_vec_chunk(1, r0, nr, 'D')
        dw_vec_evac(1, r0, nr, acc, 11, 'D')
        pss2, rows2 = dw_pe_group(1, 1)
        dw_pe_evac(1, pss2, rows2, 4)
        r0, nr = DVE_DW_ROWS[1]
        acc = dw_vec_chunk(1, r0, nr, 'D')
        dw_vec_evac(1, r0, nr, acc, 12, 'D')
        pss3, rows3 = dw_pe_group(1, 2)
        dw_pe_evac(1, pss3, rows3, 7)
        r0, nr = DVE_DW_ROWS[2]
        acc = dw_vec_chunk(1, r0, nr, 'D')
        dw_vec_evac(1, r0, nr, acc, 13, 'D')
        pss4, rows4 = dw_pe_group(1, 3)
        dw_pe_evac(1, pss4, rows4, 14)

        nc.vector.reduce_sum(out=x2sT[1][:], in_=x2s[1][:, 0:16], axis=AX)
        dgate_mark()
        gdummies(DUM_SE1)
        se_mms(1)

        # -- gate_1 + store_1 --
        for hs, pi, eng in GATE_SEQ:
            gp = gate_piece(1, hs, pi, eng)
            store_piece(1, hs, pi, gp)


# ---------------- build + run ----------------
_CACHE = {}


def _build():
    if "nc" in _CACHE:
        return _CACHE["nc"]
    nc = bacc.Bacc("TRN2", target_bir_lowering=False, debug=False,
                   enable_asserts=False, num_devices=NCORES)
    x_d = nc.dram_tensor("x_in", [BLOC, C_IN, H, W], f32, kind="ExternalInput")
    y_d = nc.dram_tensor("y_out", [BLOC, 2 * INIT, H, W], f32,
                         kind="ExternalOutput")
    cblob_d = nc.dram_tensor("cblob", [P, CBW], f32, kind="ExternalInput")
    with tile.TileContext(nc) as tc:
        _emit(tc, x_d, y_d, cblob_d)
    nc.compile()
    _CACHE["nc"] = nc
    return nc


def _run(inputs, trace=False):
    nc = _build()
    blob = _pack_consts({k: v for k, v in inputs.items() if k != "x"})
    x = np.ascontiguousarray(np.asarray(inputs["x"], dtype=np.float32))
    in_maps = []
    for ci in range(NCORES):
        in_maps.append({"x_in": np.ascontiguousarray(x[BLOC * ci:BLOC * (ci + 1)]),
                        "cblob": blob})
    res = run_bass_kernel_spmd(nc, in_maps, list(range(NCORES)), trace=trace)
    out = np.concatenate([res.results[ci]["y_out"] for ci in range(NCORES)], axis=0)
    return out, res


def kernel(**inputs):
    out, _ = _run(inputs)
    return out


# revision 5
# speedup vs baseline: 1.0093x; 1.0093x over previous
"""Trainium2 Bass kernel for nn_DCAA_57604101374115 (moe_routing).

Per-sample pipelined implementation. Each of the 8 cores gets 2 samples
(pure data parallel over batch 16). Per core, the two samples run as
pipeline units so sample 1's HBM load hides under sample 0's compute and
sample 0's store hides sample 1's compute (the DMA bus is a single serial
360 GB/s resource in the cost model; per-core traffic is 6.4 MB in +
12.8 MB out).

Per-sample layout: SBUF partitions = (image half in {top,bottom}) x
(channel 0..63); free dim = (row-in-half 0..55, col 0..111). All per-op
costs (free-size based) match the packed 2-sample layout, and the 3x3
depthwise halo at the half boundary is produced by two extra 112-col
"swap block" matmuls (weight maps half h input to half 1-h output).

Engine split per sample:
  PE   : dynamic 1x1 conv (14x448-col block-diag matmuls + 2 halo) and
         34 rows of the depthwise conv as diag-matmul PSUM accumulation.
  DVE  : 12 rows of depthwise (scalar_tensor_tensor MACs), routing/SE
         small ops, gating (tensor_scalar at the 2x SBUF rate).
  Pool : 10 rows of depthwise, memsets, some evac/gating.
  ACT  : all BN+ReLU PSUM/SBUF evacuations with accum_out providing the
         spatial sums for routing/SE, sigmoids, some gating.
PE p-state: dummy matmuls keep the tensor engine's busy-run alive across
phase gaps so real matmuls run at the full 2.4 GHz rate.
"""

import numpy as np
from contextlib import ExitStack

import concourse.bass as bass
import concourse.tile as tile
from concourse import bacc, mybir
from concourse.bass_utils import run_bass_kernel_spmd

# ---------------- problem constants ----------------
B, C_IN, H, W = 16, 64, 112, 112
INIT = 64
NEW = 64
E = 4
SE_HID = 32
EPS = 1e-5
NCORES = 8
BLOC = B // NCORES          # 2 samples per core
P = 128
HH = H // 2                 # 56 rows per half
IMG = HH * W                # 6272 elements per partition per sample
PADH, PADW = HH + 2, W + 2  # 58 x 114 padded x1
HWF = float(H * W)

f32 = mybir.dt.float32
f32r = mybir.dt.float32r
bf16 = mybir.dt.bfloat16
AX = mybir.AxisListType.X
MULT = mybir.AluOpType.mult
ADD = mybir.AluOpType.add
MAX = mybir.AluOpType.max
RELU = mybir.ActivationFunctionType.Relu
SIGM = mybir.ActivationFunctionType.Sigmoid
COPY = mybir.ActivationFunctionType.Copy

# ---------------- tuning knobs ----------------
NLOAD = 4                   # load pieces per sample (14 rows each)
DUM_A = 0                  # PE dummies during load0 (256 cols ~= 107-213 ns each)
DUM_B = 0                   # bridge r1_0 -> M1_0
DUM_C = 0                  # bridge M1_0 -> r2_0
DUM_D = 0                   # bridge r2_0 -> DW_0
DUM_SE0 = 0                # bridge DW_0 G3 -> se_0 matmuls
DUM_E = 0                   # bridge M1_1 -> r2_1
DUM_F = 0
DUM_SE1 = 0
DUMCOL = 256                # dummy matmul width (>=256 to stay 1 cyc/row)

# depthwise row split per sample: PE chunk list (rows each), DVE/Pool 2-row chunks
PE_DW = [4, 4, 4, 4, 4, 4, 4, 4, 4, 4, 3, 3]  # 46 rows
PE_DW_GROUPS = [(0, 4), (4, 3), (7, 3), (10, 2)]
DVE_DW_ROWS = [(46, 4), (50, 3), (53, 3)]  # (row0, nrows)

# M1 evacuation engine per chunk (14 chunks x 4 rows): A=ACT D=DVE P=Pool
M1_EVAC_0 = ['A', 'A', 'D', 'A', 'D', 'D', 'A', 'D', 'A', 'A', 'D', 'D', 'A', 'A']
M1_EVAC_1 = ['A', 'A', 'A', 'A', 'D', 'D', 'D', 'A', 'A', 'D', 'D', 'D', 'A', 'A']

# gate piece engine per (x1a, x1b, x2a, x2b)
GATE_ENG = ['D', 'A', 'D', 'P']

# ---------------- const blob ----------------
# column layout: [early | mid | se]
_EARLY = {"w1T": E * 64, "rw1": E, "rb1": 1, "maskE": E, "bn1b": 1}
_MID = {"i128": P, "rw2": E, "rb2": 1, "w2": E * 9, "bn2b": 1}
_SE = {"sew1a": SE_HID, "sew1b": SE_HID, "seb1": 1,
       "sew2a": P, "sew2b": P, "seb2a": 1, "seb2b": 1}
_OFF = {}
_off = 0
for _sec in (_EARLY, _MID, _SE):
    for _n, _w in _sec.items():
        _OFF[_n] = _off
        _off += _w
CBW = _off
EARLY_W = sum(_EARLY.values())
MID_W = sum(_MID.values())
SE_W = sum(_SE.values())


def _pack_consts(inp):
    n = {k: np.asarray(v, dtype=np.float32) for k, v in inp.items()}
    s1 = n["bn1_g"] / np.sqrt(n["bn1_v"] + EPS)
    s2 = n["bn2_g"] / np.sqrt(n["bn2_v"] + EPS)
    w1m = n["w1"][:, :, :, 0, 0] * s1[None, :, None]      # [E, O, I], bn1 scale folded
    w2m = n["w2"][:, :, 0].reshape(E, NEW, 9) * s2[None, :, None]

    c = {}
    # w1T[(h,ci), (e,co)] = w1m[e, co, ci]
    c["w1T"] = np.tile(w1m.transpose(2, 0, 1).reshape(C_IN, E * 64), (2, 1))
    c["rw1"] = np.tile(n["rw1"].T / HWF, (2, 1))          # [P, E]
    c["rb1"] = n["rb1"][:, None]                          # [E, 1]
    c["maskE"] = np.eye(E, dtype=np.float32)
    c["bn1b"] = np.tile(n["bn1_b"] - n["bn1_m"] * s1, 2)[:, None]
    c["i128"] = np.eye(P, dtype=np.float32)
    c["rw2"] = np.tile(n["rw2"].T / HWF, (2, 1))
    c["rb2"] = n["rb2"][:, None]
    c["w2"] = np.tile(w2m.transpose(1, 0, 2).reshape(NEW, E * 9), (2, 1))
    c["bn2b"] = np.tile(n["bn2_b"] - n["bn2_m"] * s2, 2)[:, None]
    c["sew1a"] = np.tile(n["se_w1"][:, :64].T / HWF, (2, 1))
    c["sew1b"] = np.tile(n["se_w1"][:, 64:].T / HWF, (2, 1))
    c["seb1"] = n["se_b1"][:, None]
    c["sew2a"] = np.zeros((SE_HID, P), np.float32)
    c["sew2a"][:, :64] = n["se_w2"][:64].T
    c["sew2a"][:, 64:] = n["se_w2"][:64].T
    c["sew2b"] = np.zeros((SE_HID, P), np.float32)
    c["sew2b"][:, :64] = n["se_w2"][64:].T
    c["sew2b"][:, 64:] = n["se_w2"][64:].T
    c["seb2a"] = np.tile(n["se_b2"][:64], 2)[:, None]
    c["seb2b"] = np.tile(n["se_b2"][64:], 2)[:, None]

    blob = np.zeros((P, CBW), np.float32)
    for sec in (_EARLY, _MID, _SE):
        for name, w in sec.items():
            v = c[name]
            blob[:v.shape[0], _OFF[name]:_OFF[name] + w] = v
    return blob


# ---------------- device kernel ----------------
def _emit(tc, x_d, y_d, cblob_d):
    nc = tc.nc
    with ExitStack() as ctx:
        const = ctx.enter_context(tc.tile_pool(name="const", bufs=1))
        data = ctx.enter_context(tc.tile_pool(name="data", bufs=1))
        small = ctx.enter_context(tc.tile_pool(name="small", bufs=1))
        stage = ctx.enter_context(tc.tile_pool(name="stage", bufs=1))
        psum = ctx.enter_context(tc.tile_pool(name="psum", bufs=1, space="PSUM"))

        cblob = const.tile([P, CBW], f32)
        ct = {}
        for sec in (_EARLY, _MID, _SE):
            for name, w in sec.items():
                rows = {"rb1": E, "maskE": E, "rb2": E, "seb1": SE_HID,
                        "sew2a": SE_HID, "sew2b": SE_HID}.get(name, P)
                ct[name] = cblob[0:rows, _OFF[name]:_OFF[name] + w]

        # ---- DMA stream (SP): consts + loads; stores are emitted later ----
        nc.sync.dma_start(cblob[:, 0:EARLY_W], cblob_d.ap()[:, 0:EARLY_W])
        x_ap = x_d.ap().rearrange("b c r w -> b c (r w)")
        xb, xr, x1pad = [], [], []
        for s in range(BLOC):
            xb.append(data.tile([P, IMG], f32, name=f"xb{s}"))
            xr.append(data.tile([P, IMG], f32r, name=f"xr{s}"))
            x1pad.append(data.tile([P, PADH * PADW], f32r, name=f"x1pad{s}"))
        LP = IMG // NLOAD

        def load_piece(s, i):
            for hf in range(2):
                nc.sync.dma_start(
                    xb[s][64 * hf:64 * hf + 64, LP * i:LP * (i + 1)],
                    x_ap[s, :, HH * W * hf + LP * i:HH * W * hf + LP * (i + 1)])

        for i in range(NLOAD):
            load_piece(0, i)
        nc.sync.dma_start(cblob[:, EARLY_W:EARLY_W + MID_W],
                          cblob_d.ap()[:, EARLY_W:EARLY_W + MID_W])
        for i in range(NLOAD):
            load_piece(1, i)
        nc.sync.dma_start(cblob[:, EARLY_W + MID_W:CBW],
                          cblob_d.ap()[:, EARLY_W + MID_W:CBW])

        xrv = [xr[s].rearrange("p (r w) -> p r w", w=W) for s in range(BLOC)]
        x1v = [x1pad[s].rearrange("p (r w) -> p r w", w=PADW) for s in range(BLOC)]
        # depthwise outputs reuse the landing buffers (dead after the casts)
        x2v = [xb[0].rearrange("p (r w) -> p r w", w=W),
               xb[1].rearrange("p (r w) -> p r w", w=W)]

        # ---- small tiles ----
        def sm(shape, nm, dt=f32):
            return small.tile(shape, dt, name=nm)
        xsum = [sm([P, NLOAD], f"xsum{s}") for s in range(2)]
        x1s = [sm([P, 16], f"x1s{s}") for s in range(2)]
        x2s = [sm([P, 24], f"x2s{s}") for s in range(2)]
        xsumT = [sm([P, 1], f"xsumT{s}") for s in range(2)]
        x1sT = [sm([P, 1], f"x1sT{s}") for s in range(2)]
        x2sT = [sm([P, 1], f"x2sT{s}") for s in range(2)]
        r1s = [sm([E, 1], f"r1s{s}") for s in range(2)]
        r2s = [sm([E, 1], f"r2s{s}") for s in range(2)]
        r1m = [sm([E, E], f"r1m{s}") for s in range(2)]
        r2m = [sm([E, E], f"r2m{s}") for s in range(2)]
        r1b = [sm([P, E], f"r1b{s}") for s in range(2)]
        r2b = [sm([P, E], f"r2b{s}") for s in range(2)]
        k1c = [sm([P, 64], f"k1c{s}") for s in range(2)]
        k2c = [sm([P, 9], f"k2c{s}") for s in range(2)]
        mm1w = [sm([P, P], f"mm1w{s}", f32r) for s in range(2)]
        mm1sw = [sm([P, P], f"mm1sw{s}", f32r) for s in range(2)]
        dwt = [sm([P, 9 * P], f"dwt{s}", f32r) for s in range(2)]
        seh = [sm([SE_HID, 1], f"seh{s}") for s in range(2)]
        s1c = [sm([P, 1], f"s1c{s}") for s in range(2)]
        s2c = [sm([P, 1], f"s2c{s}") for s in range(2)]
        ones4 = sm([E, P], "ones4")
        warm = sm([1, 1], "warm")

        # ---- ACT table warm + structural zeroing (all idle time).
        # f32r tiles cannot be memset; zero them with x0.0 compute writes
        # from an already-loaded const region (waits the early-const DMA).
        nc.scalar.activation(warm[:], cblob[0:1, 0:1], SIGM, bias=0.0, scale=1.0)
        nc.gpsimd.memset(ones4[:], 1.0)
        zsrc = cblob[:, 0:PADW]
        for s in range(BLOC):
            nc.vector.tensor_scalar_mul(mm1w[s][:], cblob[:, 0:P], 0.0)
            nc.vector.tensor_scalar_mul(mm1sw[s][:], cblob[:, 0:P], 0.0)
            nc.vector.tensor_scalar_mul(x1v[s][:, 0, :], zsrc[:, 0:PADW], 0.0)
            nc.vector.tensor_scalar_mul(x1v[s][:, PADH - 1, :], zsrc[:, 0:PADW], 0.0)
            nc.vector.tensor_scalar_mul(x1v[s][:, :, 0], zsrc[:, 0:PADH], 0.0)
            nc.vector.tensor_scalar_mul(x1v[s][:, :, PADW - 1], zsrc[:, 0:PADH], 0.0)

        # ---- helpers ----
        dum_ps = psum.tile([P, DUMCOL], f32, tag="dum", bufs=1)
        dum_src = small.tile([P, DUMCOL], f32r, name="dum_src")
        dgate = small.tile([P, DUMCOL], f32r, name="dgate")
        nc.vector.tensor_scalar_mul(dum_src[:], cblob[:, 0:DUMCOL], 0.0)
        nc.vector.tensor_scalar_mul(dgate[:], cblob[:, 0:DUMCOL], 0.0)
        dum_lhs = dum_src[:, 0:P]

        def dummies(n):
            # ungated: ready as soon as dum_src exists
            for _ in range(n):
                nc.tensor.matmul(dum_ps[:], dum_lhs, dum_src[:, 0:DUMCOL],
                                 start=True, stop=True)

        def dgate_mark():
            # rewrite the gate tile on DVE at this point in its stream; the
            # next gated-dummy batch becomes ready only once this runs
            nc.vector.tensor_scalar_mul(dgate[:], cblob[:, 0:DUMCOL], 0.0)

        def gdummies(n):
            # gated: wait the latest dgate version, so the scheduler cannot
            # hoist these bridge dummies ahead of their phase
            for _ in range(n):
                nc.tensor.matmul(dum_ps[:], dum_lhs, dgate[:, 0:DUMCOL],
                                 start=True, stop=True)

        def xsum_partial(s, i, eng):
            # fp32 -> fp32r rounding cast + spatial-sum side channel
            src = xb[s][:, LP * i:LP * (i + 1)]
            dst = xr[s][:, LP * i:LP * (i + 1)]
            if eng == 'A':
                nc.scalar.activation(dst, src, COPY, bias=0.0, scale=1.0,
                                     accum_out=xsum[s][:, i:i + 1])
            else:
                nc.vector.tensor_scalar(out=dst, in0=src, scalar1=1.0,
                                        scalar2=None, op0=MULT, op1=ADD,
                                        accum_out=xsum[s][:, i:i + 1])

        def r1_mm_pre(s):
            ps = psum.tile([E, 1], f32, tag="sm", bufs=1, name="smps")
            nc.tensor.matmul(ps[:], ct["rw1"], xsumT[s][:], start=True, stop=True)
            return ps

        def r_chain_smalls(s, pre_ps, rs, rm, rb_, kc, wsrc, ncols, rbias):
            # ACT sigmoid; DVE mask-mul; PE bcast matmul; DVE copy + mix
            nc.scalar.activation(rs[:], pre_ps[:], SIGM, bias=rbias, scale=1.0)
            nc.vector.tensor_scalar_mul(rm[:], ct["maskE"], rs[:, 0:1])
            bp = psum.tile([P, E], f32, tag="sm", bufs=1, name="smps")
            nc.tensor.matmul(bp[:], ones4[:], rm[:], start=True, stop=True)
            nc.vector.tensor_copy(rb_[:], bp[:])
            nc.vector.tensor_scalar_mul(kc[:], wsrc[:, 0:ncols], rb_[:, 0:1])
            for e in range(1, E):
                nc.vector.scalar_tensor_tensor(
                    kc[:], wsrc[:, e * ncols:(e + 1) * ncols],
                    rb_[:, e:e + 1], kc[:], op0=MULT, op1=ADD)

        def build_mm1(s):
            nc.vector.tensor_scalar_mul(mm1w[s][0:64, 0:64], k1c[s][0:64, :], 1.0)
            nc.vector.tensor_scalar_mul(mm1w[s][64:128, 64:128], k1c[s][64:128, :], 1.0)
            nc.vector.tensor_scalar_mul(mm1sw[s][0:64, 64:128], k1c[s][0:64, :], 1.0)
            nc.vector.tensor_scalar_mul(mm1sw[s][64:128, 0:64], k1c[s][64:128, :], 1.0)

        def m1_chunk(s, g):
            ps = psum.tile([P, 448], f32, tag="mm", bufs=6, name="mmps")
            nc.tensor.matmul(ps[:], mm1w[s][:],
                             xrv[s][:, 4 * g:4 * g + 4, :],
                             start=True, stop=True)
            return ps

        def m1_evac(s, g, ps, eng):
            dst = x1v[s][:, 1 + 4 * g:1 + 4 * g + 4, 1:1 + W]
            src = ps[:, 0:448].rearrange("p (r w) -> p r w", w=W)
            acc = x1s[s][:, g:g + 1]
            if eng == 'A':
                nc.scalar.activation(dst, src, RELU, bias=ct["bn1b"], scale=1.0,
                                     accum_out=acc)
            else:
                # two ops: bias+relu, then in-place copy carrying accum_out
                # (DVE two-scalar ts with accum_out mis-executes on HW)
                nc.vector.tensor_scalar(out=dst, in0=src, scalar1=ct["bn1b"],
                                        scalar2=0.0, op0=ADD, op1=MAX)
                nc.vector.tensor_scalar(out=dst, in0=dst.bitcast(f32),
                                        scalar1=1.0, scalar2=None,
                                        op0=MULT, op1=ADD, accum_out=acc)

        def m1_halo(s):
            # swap-block conv of half-boundary rows -> pad halo rows
            h0 = psum.tile([P, 112], f32, tag="mm", bufs=6, name="mmps")
            nc.tensor.matmul(h0[:], mm1sw[s][:], xrv[s][:, 0, :],
                             start=True, stop=True)
            nc.scalar.activation(x1v[s][0:64, PADH - 1, 1:1 + W], h0[0:64, :],
                                 RELU, bias=ct["bn1b"][0:64, :], scale=1.0)
            h1 = psum.tile([P, 112], f32, tag="mm", bufs=6, name="mmps")
            nc.tensor.matmul(h1[:], mm1sw[s][:], xrv[s][:, HH - 1, :],
                             start=True, stop=True)
            nc.scalar.activation(x1v[s][64:128, 0, 1:1 + W], h1[64:128, :],
                                 RELU, bias=ct["bn1b"][64:128, :], scale=1.0)

        def build_dwt(s):
            for t in range(9):
                nc.vector.tensor_scalar_mul(dwt[s][:, t * P:(t + 1) * P],
                                            ct["i128"], k2c[s][:, t:t + 1])

        def dw_pe_group(s, gi):
            c0, nch = PE_DW_GROUPS[gi]
            pss = []
            rows = []
            for ci in range(nch):
                nr = PE_DW[c0 + ci]
                r0 = sum(PE_DW[:c0 + ci])
                pss.append(psum.tile([P, 448], f32, tag="mm", bufs=6, name="mmps"))
                rows.append((r0, nr))
            for t in range(9):
                dy, dx = divmod(t, 3)
                for ci in range(nch):
                    r0, nr = rows[ci]
                    nc.tensor.matmul(
                        pss[ci][:, 0:nr * W],
                        dwt[s][:, t * P:(t + 1) * P],
                        x1v[s][:, r0 + dy:r0 + dy + nr, dx:dx + W],
                        start=(t == 0), stop=(t == 8))
            return pss, rows

        def dw_pe_evac(s, pss, rows, cols):
            for ci, (ps, (r0, nr)) in enumerate(zip(pss, rows)):
                nc.scalar.activation(
                    x2v[s][:, r0:r0 + nr, :],
                    ps[:, 0:nr * W].rearrange("p (r w) -> p r w", w=W),
                    RELU, bias=ct["bn2b"], scale=1.0,
                    accum_out=x2s[s][:, cols + ci:cols + ci + 1])

        def dw_vec_chunk(s, r0, nr, eng):
            acc = stage.tile([P, 448], f32, tag="acc" + eng, bufs=3, name="acc")
            accv = acc[:, 0:nr * W].rearrange("p (r w) -> p r w", w=W)
            e = nc.vector if eng == 'D' else nc.gpsimd
            e.tensor_scalar(out=accv, in0=x1v[s][:, r0:r0 + nr, 0:W].bitcast(f32),
                            scalar1=k2c[s][:, 0:1], scalar2=ct["bn2b"],
                            op0=MULT, op1=ADD)
            for t in range(1, 9):
                dy, dx = divmod(t, 3)
                e.scalar_tensor_tensor(
                    accv, x1v[s][:, r0 + dy:r0 + dy + nr, dx:dx + W].bitcast(f32),
                    k2c[s][:, t:t + 1], accv, op0=MULT, op1=ADD)
            return acc

        def dw_pool_chunk(s, r0, nr):
            # Pool lacks scalar_tensor_tensor: tap product via tensor_scalar
            # into a temp, accumulate via tensor_tensor add.
            acc = stage.tile([P, 448], f32, tag="accP", bufs=2, name="acc")
            tmp = stage.tile([P, 448], f32, tag="tmpP", bufs=2, name="tmp")
            accv = acc[:, 0:nr * W].rearrange("p (r w) -> p r w", w=W)
            tmpv = tmp[:, 0:nr * W].rearrange("p (r w) -> p r w", w=W)
            nc.gpsimd.tensor_scalar(out=accv,
                                    in0=x1v[s][:, r0:r0 + nr, 0:W].bitcast(f32),
                                    scalar1=k2c[s][:, 0:1], scalar2=ct["bn2b"],
                                    op0=MULT, op1=ADD)
            for t in range(1, 9):
                dy, dx = divmod(t, 3)
                nc.gpsimd.tensor_scalar_mul(
                    tmpv, x1v[s][:, r0 + dy:r0 + dy + nr, dx:dx + W].bitcast(f32),
                    k2c[s][:, t:t + 1])
                nc.gpsimd.tensor_tensor(out=accv, in0=accv, in1=tmpv, op=ADD)
            return acc

        def dw_vec_evac(s, r0, nr, acc, col, eng):
            # relu into acc (bias folded into tap 0), then copy acc -> x2
            # carrying the accum_out side-channel (baseline-proven shapes)
            e = nc.vector if eng == 'D' else nc.gpsimd
            accv = acc[:, 0:nr * W].rearrange("p (r w) -> p r w", w=W)
            e.tensor_scalar(out=accv, in0=accv, scalar1=0.0, scalar2=0.0,
                            op0=MAX, op1=ADD)
            e.tensor_scalar(out=x2v[s][:, r0:r0 + nr, :], in0=accv,
                            scalar1=1.0, scalar2=None, op0=MULT, op1=ADD,
                            accum_out=x2s[s][:, col:col + 1])

        def se_mms(s):
            se1 = psum.tile([SE_HID, 1], f32, tag="sm", bufs=1, name="smps")
            nc.tensor.matmul(se1[:], ct["sew1a"], x1sT[s][:], start=True, stop=False)
            nc.tensor.matmul(se1[:], ct["sew1b"], x2sT[s][:], start=False, stop=True)
            nc.scalar.activation(seh[s][:], se1[:], RELU, bias=ct["seb1"], scale=1.0)
            g1 = psum.tile([P, 1], f32, tag="sm", bufs=1, name="smps")
            nc.tensor.matmul(g1[:], ct["sew2a"], seh[s][:], start=True, stop=True)
            nc.scalar.activation(s1c[s][:], g1[:], SIGM, bias=ct["seb2a"], scale=1.0)
            g2 = psum.tile([P, 1], f32, tag="sm", bufs=1, name="smps")
            nc.tensor.matmul(g2[:], ct["sew2b"], seh[s][:], start=True, stop=True)
            nc.scalar.activation(s2c[s][:], g2[:], SIGM, bias=ct["seb2b"], scale=1.0)

        GR = 14                  # gate/store piece rows

        def gate_piece(s, half_src, pi, eng):
            # half_src: 0 -> x1 (from x1pad interior), 1 -> x2
            r0 = GR * pi
            if half_src == 0:
                src = x1v[s][:, 1 + r0:1 + r0 + GR, 1:1 + W].bitcast(f32)
                sc = s1c[s][:, 0:1]
            else:
                src = x2v[s][:, r0:r0 + GR, :]
                sc = s2c[s][:, 0:1]
            nbuf = {'D': 2, 'A': 1, 'P': 2}[eng]
            gst = stage.tile([P, GR * W], f32, tag="gst" + eng, bufs=nbuf,
                             name="gst")
            gv = gst.rearrange("p (r w) -> p r w", w=W)
            if eng == 'D':
                nc.vector.tensor_scalar_mul(gv, src, sc)
            elif eng == 'P':
                nc.gpsimd.tensor_scalar_mul(gv, src, sc)
            else:
                nc.scalar.activation(gv, src, COPY, bias=0.0, scale=sc)
            return gst

        y_ap1 = y_d.ap()[:, 0:64].rearrange("b c r w -> b c (r w)")
        y_ap2 = y_d.ap()[:, 64:128].rearrange("b c r w -> b c (r w)")

        def store_piece(s, half_src, pi, gst):
            dst = (y_ap1 if half_src == 0 else y_ap2)
            for hf in range(2):
                off = HH * W * hf + GR * W * pi
                nc.sync.dma_start(dst[s, :, off:off + GR * W],
                                  gst[64 * hf:64 * hf + 64, :])

        # ================= emission schedule =================
        # -- load0 window --
        dummies(DUM_A)
        xsum_partial(0, 0, 'A')
        xsum_partial(0, 1, 'A')
        xsum_partial(0, 2, 'A')
        xsum_partial(0, 3, 'D')
        nc.vector.reduce_sum(out=xsumT[0][:], in_=xsum[0][:, 0:NLOAD], axis=AX)
        pre0 = r1_mm_pre(0)
        r_chain_smalls(0, pre0, r1s[0], r1m[0], r1b[0], k1c[0], ct["w1T"], 64,
                       ct["rb1"])
        build_mm1(0)
        dgate_mark()
        gdummies(DUM_B)

        # -- M1_0 --
        for g in range(14):
            ps = m1_chunk(0, g)
            m1_evac(0, g, ps, M1_EVAC_0[g])
        m1_halo(0)
        nc.vector.reduce_sum(out=x1sT[0][:], in_=x1s[0][:, 0:14], axis=AX)
        dgate_mark()
        gdummies(DUM_C)

        # -- r2_0 --
        pre = psum.tile([E, 1], f32, tag="sm", bufs=1, name="smps")
        nc.tensor.matmul(pre[:], ct["rw2"], x1sT[0][:], start=True, stop=True)
        r_chain_smalls(0, pre, r2s[0], r2m[0], r2b[0], k2c[0], ct["w2"], 9,
                       ct["rb2"])
        build_dwt(0)
        dgate_mark()
        gdummies(DUM_D)

        # -- DW_0 + sample-1 prep interleave --
        xsum_partial(1, 0, 'A')
        xsum_partial(1, 1, 'A')
        xsum_partial(1, 2, 'D')
        xsum_partial(1, 3, 'D')
        nc.vector.reduce_sum(out=xsumT[1][:], in_=xsum[1][:, 0:NLOAD], axis=AX)
        pss1, rows1 = dw_pe_group(0, 0)  # PE G1 (4 chunks)
        dw_pe_evac(0, pss1, rows1, 0)

        # r1_1 chain (PE mms slot in mid-DW stream)
        pre1 = r1_mm_pre(1)
        r_chain_smalls(1, pre1, r1s[1], r1m[1], r1b[1], k1c[1], ct["w1T"], 64,
                       ct["rb1"])
        build_mm1(1)

        # dve DW chunks 0-1
        r0, nr = DVE_DW_ROWS[0]
        acc = dw_vec_chunk(0, r0, nr, 'D')
        dw_vec_evac(0, r0, nr, acc, 11, 'D')

        pss2, rows2 = dw_pe_group(0, 1)  # PE G2 (3 chunks)
        dw_pe_evac(0, pss2, rows2, 4)
        r0, nr = DVE_DW_ROWS[1]
        acc = dw_vec_chunk(0, r0, nr, 'D')
        dw_vec_evac(0, r0, nr, acc, 12, 'D')
        pss3, rows3 = dw_pe_group(0, 2)  # PE G3
        dw_pe_evac(0, pss3, rows3, 7)
        r0, nr = DVE_DW_ROWS[2]
        acc = dw_vec_chunk(0, r0, nr, 'D')
        dw_vec_evac(0, r0, nr, acc, 13, 'D')
        pss4, rows4 = dw_pe_group(0, 3)  # PE G4
        dw_pe_evac(0, pss4, rows4, 14)

        nc.vector.reduce_sum(out=x2sT[0][:], in_=x2s[0][:, 0:16], axis=AX)
        dgate_mark()
        gdummies(DUM_SE0)
        se_mms(0)

        # -- gate_0 pieces + M1_1 --
        GATE_SEQ = [(0, 0, 'D'), (0, 1, 'P'), (0, 2, 'D'), (0, 3, 'P'),
                    (1, 0, 'D'), (1, 1, 'P'), (1, 2, 'D'), (1, 3, 'P')]
        gp = gate_piece(0, *GATE_SEQ[0])
        store_piece(0, GATE_SEQ[0][0], GATE_SEQ[0][1], gp)
        gp = gate_piece(0, *GATE_SEQ[1])
        store_piece(0, GATE_SEQ[1][0], GATE_SEQ[1][1], gp)
        for g in range(14):
            ps = m1_chunk(1, g)
            m1_evac(1, g, ps, M1_EVAC_1[g])
        m1_halo(1)
        for hs, pi, eng in GATE_SEQ[2:]:
            gp = gate_piece(0, hs, pi, eng)
            store_piece(0, hs, pi, gp)
        nc.vector.reduce_sum(out=x1sT[1][:], in_=x1s[1][:, 0:14], axis=AX)
        dgate_mark()
        gdummies(DUM_E)

        # -- r2_1 --
        pre = psum.tile([E, 1], f32, tag="sm", bufs=1, name="smps")
        nc.tensor.matmul(pre[:], ct["rw2"], x1sT[1][:], start=True, stop=True)
        r_chain_smalls(1, pre, r2s[1], r2m[1], r2b[1], k2c[1], ct["w2"], 9,
                       ct["rb2"])
        build_dwt(1)
        dgate_mark()
        gdummies(DUM_F)

        # -- DW_1 --
        pss1, rows1 = dw_pe_group(1, 0)
        dw_pe_evac(1, pss1, rows1, 0)
        r0, nr = DVE_DW_ROWS[0]
        acc = dw_vec_chunk(1, r0, nr, 'D')
        dw_vec_evac(1, r0, nr, acc, 11, 'D')
        pss2, rows2 = dw_pe_group(1, 1)
        dw_pe_evac(1, pss2, rows2, 4)
        r0, nr = DVE_DW_ROWS[1]
        acc = dw_vec_chunk(1, r0, nr, 'D')
        dw_vec_evac(1, r0, nr, acc, 12, 'D')
        pss3, rows3 = dw_pe_group(1, 2)
        dw_pe_evac(1, pss3, rows3, 7)
        r0, nr = DVE_DW_ROWS[2]
        acc = dw_vec_chunk(1, r0, nr, 'D')
        dw_vec_evac(1, r0, nr, acc, 13, 'D')
        pss4, rows4 = dw_pe_group(1, 3)
        dw_pe_evac(1, pss4, rows4, 14)

        nc.vector.reduce_sum(out=x2sT[1][:], in_=x2s[1][:, 0:16], axis=AX)
        dgate_mark()
        gdummies(DUM_SE1)
        se_mms(1)

        # -- gate_1 + store_1 --
        for hs, pi, eng in GATE_SEQ:
            gp = gate_piece(1, hs, pi, eng)
            store_piece(1, hs, pi, gp)


# ---------------- build + run ----------------
_CACHE = {}


def _build():
    if "nc" in _CACHE:
        return _CACHE["nc"]
    nc = bacc.Bacc("TRN2", target_bir_lowering=False, debug=False,
                   enable_asserts=False, num_devices=NCORES)
    x_d = nc.dram_tensor("x_in", [BLOC, C_IN, H, W], f32, kind="ExternalInput")
    y_d = nc.dram_tensor("y_out", [BLOC, 2 * INIT, H, W], f32,
                         kind="ExternalOutput")
    cblob_d = nc.dram_tensor("cblob", [P, CBW], f32, kind="ExternalInput")
    with tile.TileContext(nc) as tc:
        _emit(tc, x_d, y_d, cblob_d)
    nc.compile()
    _CACHE["nc"] = nc
    return nc


def _run(inputs, trace=False):
    nc = _build()
    blob = _pack_consts({k: v for k, v in inputs.items() if k != "x"})
    x = np.ascontiguousarray(np.asarray(inputs["x"], dtype=np.float32))
    in_maps = []
    for ci in range(NCORES):
        in_maps.append({"x_in": np.ascontiguousarray(x[BLOC * ci:BLOC * (ci + 1)]),
                        "cblob": blob})
    res = run_bass_kernel_spmd(nc, in_maps, list(range(NCORES)), trace=trace)
    out = np.concatenate([res.results[ci]["y_out"] for ci in range(NCORES)], axis=0)
    return out, res


def kernel(**inputs):
    out, _ = _run(inputs)
    return out


# revision 6
# speedup vs baseline: 1.0130x; 1.0036x over previous
"""Trainium2 Bass kernel for nn_DCAA_57604101374115 (moe_routing).

Per-sample pipelined implementation. Each of the 8 cores gets 2 samples
(pure data parallel over batch 16). Per core, the two samples run as
pipeline units so sample 1's HBM load hides under sample 0's compute and
sample 0's store hides sample 1's compute (the DMA bus is a single serial
360 GB/s resource in the cost model; per-core traffic is 6.4 MB in +
12.8 MB out).

Per-sample layout: SBUF partitions = (image half in {top,bottom}) x
(channel 0..63); free dim = (row-in-half 0..55, col 0..111). All per-op
costs (free-size based) match the packed 2-sample layout, and the 3x3
depthwise halo at the half boundary is produced by two extra 112-col
"swap block" matmuls (weight maps half h input to half 1-h output).

Engine split per sample:
  PE   : dynamic 1x1 conv (14x448-col block-diag matmuls + 2 halo) and
         34 rows of the depthwise conv as diag-matmul PSUM accumulation.
  DVE  : 12 rows of depthwise (scalar_tensor_tensor MACs), routing/SE
         small ops, gating (tensor_scalar at the 2x SBUF rate).
  Pool : 10 rows of depthwise, memsets, some evac/gating.
  ACT  : all BN+ReLU PSUM/SBUF evacuations with accum_out providing the
         spatial sums for routing/SE, sigmoids, some gating.
PE p-state: dummy matmuls keep the tensor engine's busy-run alive across
phase gaps so real matmuls run at the full 2.4 GHz rate.
"""

import numpy as np
from contextlib import ExitStack

import concourse.bass as bass
import concourse.tile as tile
from concourse import bacc, mybir
from concourse.bass_utils import run_bass_kernel_spmd

# ---------------- problem constants ----------------
B, C_IN, H, W = 16, 64, 112, 112
INIT = 64
NEW = 64
E = 4
SE_HID = 32
EPS = 1e-5
NCORES = 8
BLOC = B // NCORES          # 2 samples per core
P = 128
HH = H // 2                 # 56 rows per half
IMG = HH * W                # 6272 elements per partition per sample
PADH, PADW = HH + 2, W + 2  # 58 x 114 padded x1
HWF = float(H * W)

f32 = mybir.dt.float32
f32r = mybir.dt.float32r
bf16 = mybir.dt.bfloat16
AX = mybir.AxisListType.X
MULT = mybir.AluOpType.mult
ADD = mybir.AluOpType.add
MAX = mybir.AluOpType.max
RELU = mybir.ActivationFunctionType.Relu
SIGM = mybir.ActivationFunctionType.Sigmoid
COPY = mybir.ActivationFunctionType.Copy

# ---------------- tuning knobs ----------------
NLOAD = 4                   # load pieces per sample (14 rows each)
DUM_A = 0                  # PE dummies during load0 (256 cols ~= 107-213 ns each)
DUM_B = 0                   # bridge r1_0 -> M1_0
DUM_C = 0                  # bridge M1_0 -> r2_0
DUM_D = 0                   # bridge r2_0 -> DW_0
DUM_SE0 = 0                # bridge DW_0 G3 -> se_0 matmuls
DUM_E = 0                   # bridge M1_1 -> r2_1
DUM_F = 0
DUM_SE1 = 0
DUMCOL = 256                # dummy matmul width (>=256 to stay 1 cyc/row)

# depthwise row split per sample: PE chunk list (rows each), DVE/Pool 2-row chunks
PE_DW = [4, 4, 4, 4, 4, 4, 4, 4, 4, 4, 3, 3]  # 46 rows
PE_DW_GROUPS = [(0, 4), (4, 3), (7, 3), (10, 2)]
DVE_DW_ROWS = [(46, 4), (50, 3), (53, 3)]  # (row0, nrows)

# M1 evacuation engine per chunk (14 chunks x 4 rows): A=ACT D=DVE P=Pool
M1_EVAC_0 = ['A', 'A', 'D', 'A', 'D', 'D', 'A', 'D', 'A', 'A', 'D', 'D', 'A', 'A']
M1_EVAC_1 = ['A', 'A', 'A', 'A', 'D', 'D', 'D', 'A', 'A', 'D', 'D', 'D', 'A', 'A']

# gate piece engine per (x1a, x1b, x2a, x2b)
GATE_ENG = ['D', 'A', 'D', 'P']

# ---------------- const blob ----------------
# column layout: [early | mid | se]
_EARLY = {"w1T": E * 64, "rw1": E, "rb1": 1, "maskE": E, "bn1b": 1}
_MID = {"i128": P, "rw2": E, "rb2": 1, "w2": E * 9, "bn2b": 1}
_SE = {"sew1a": SE_HID, "sew1b": SE_HID, "seb1": 1,
       "sew2a": P, "sew2b": P, "seb2a": 1, "seb2b": 1}
_OFF = {}
_off = 0
for _sec in (_EARLY, _MID, _SE):
    for _n, _w in _sec.items():
        _OFF[_n] = _off
        _off += _w
CBW = _off
EARLY_W = sum(_EARLY.values())
MID_W = sum(_MID.values())
SE_W = sum(_SE.values())


def _pack_consts(inp):
    n = {k: np.asarray(v, dtype=np.float32) for k, v in inp.items()}
    s1 = n["bn1_g"] / np.sqrt(n["bn1_v"] + EPS)
    s2 = n["bn2_g"] / np.sqrt(n["bn2_v"] + EPS)
    w1m = n["w1"][:, :, :, 0, 0] * s1[None, :, None]      # [E, O, I], bn1 scale folded
    w2m = n["w2"][:, :, 0].reshape(E, NEW, 9) * s2[None, :, None]

    c = {}
    # w1T[(h,ci), (e,co)] = w1m[e, co, ci]
    c["w1T"] = np.tile(w1m.transpose(2, 0, 1).reshape(C_IN, E * 64), (2, 1))
    c["rw1"] = np.tile(n["rw1"].T / HWF, (2, 1))          # [P, E]
    c["rb1"] = n["rb1"][:, None]                          # [E, 1]
    c["maskE"] = np.eye(E, dtype=np.float32)
    c["bn1b"] = np.tile(n["bn1_b"] - n["bn1_m"] * s1, 2)[:, None]
    c["i128"] = np.eye(P, dtype=np.float32)
    c["rw2"] = np.tile(n["rw2"].T / HWF, (2, 1))
    c["rb2"] = n["rb2"][:, None]
    c["w2"] = np.tile(w2m.transpose(1, 0, 2).reshape(NEW, E * 9), (2, 1))
    c["bn2b"] = np.tile(n["bn2_b"] - n["bn2_m"] * s2, 2)[:, None]
    c["sew1a"] = np.tile(n["se_w1"][:, :64].T / HWF, (2, 1))
    c["sew1b"] = np.tile(n["se_w1"][:, 64:].T / HWF, (2, 1))
    c["seb1"] = n["se_b1"][:, None]
    c["sew2a"] = np.zeros((SE_HID, P), np.float32)
    c["sew2a"][:, :64] = n["se_w2"][:64].T
    c["sew2a"][:, 64:] = n["se_w2"][:64].T
    c["sew2b"] = np.zeros((SE_HID, P), np.float32)
    c["sew2b"][:, :64] = n["se_w2"][64:].T
    c["sew2b"][:, 64:] = n["se_w2"][64:].T
    c["seb2a"] = np.tile(n["se_b2"][:64], 2)[:, None]
    c["seb2b"] = np.tile(n["se_b2"][64:], 2)[:, None]

    blob = np.zeros((P, CBW), np.float32)
    for sec in (_EARLY, _MID, _SE):
        for name, w in sec.items():
            v = c[name]
            blob[:v.shape[0], _OFF[name]:_OFF[name] + w] = v
    return blob


# ---------------- device kernel ----------------
def _emit(tc, x_d, y_d, cblob_d):
    nc = tc.nc
    with ExitStack() as ctx:
        const = ctx.enter_context(tc.tile_pool(name="const", bufs=1))
        data = ctx.enter_context(tc.tile_pool(name="data", bufs=1))
        small = ctx.enter_context(tc.tile_pool(name="small", bufs=1))
        stage = ctx.enter_context(tc.tile_pool(name="stage", bufs=1))
        psum = ctx.enter_context(tc.tile_pool(name="psum", bufs=1, space="PSUM"))

        cblob = const.tile([P, CBW], f32)
        ct = {}
        for sec in (_EARLY, _MID, _SE):
            for name, w in sec.items():
                rows = {"rb1": E, "maskE": E, "rb2": E, "seb1": SE_HID,
                        "sew2a": SE_HID, "sew2b": SE_HID}.get(name, P)
                ct[name] = cblob[0:rows, _OFF[name]:_OFF[name] + w]

        # ---- DMA stream (SP): consts + loads; stores are emitted later ----
        nc.sync.dma_start(cblob[:, 0:EARLY_W], cblob_d.ap()[:, 0:EARLY_W])
        x_ap = x_d.ap().rearrange("b c r w -> b c (r w)")
        xb, xr, x1pad = [], [], []
        for s in range(BLOC):
            xb.append(data.tile([P, IMG], f32, name=f"xb{s}"))
            xr.append(data.tile([P, IMG], f32r, name=f"xr{s}"))
            x1pad.append(data.tile([P, PADH * PADW], f32r, name=f"x1pad{s}"))
        LP = IMG // NLOAD

        def load_piece(s, i):
            for hf in range(2):
                nc.sync.dma_start(
                    xb[s][64 * hf:64 * hf + 64, LP * i:LP * (i + 1)],
                    x_ap[s, :, HH * W * hf + LP * i:HH * W * hf + LP * (i + 1)])

        for i in range(NLOAD):
            load_piece(0, i)
        nc.sync.dma_start(cblob[:, EARLY_W:EARLY_W + MID_W],
                          cblob_d.ap()[:, EARLY_W:EARLY_W + MID_W])
        for i in range(NLOAD):
            load_piece(1, i)
        nc.sync.dma_start(cblob[:, EARLY_W + MID_W:CBW],
                          cblob_d.ap()[:, EARLY_W + MID_W:CBW])

        xrv = [xr[s].rearrange("p (r w) -> p r w", w=W) for s in range(BLOC)]
        x1v = [x1pad[s].rearrange("p (r w) -> p r w", w=PADW) for s in range(BLOC)]
        # depthwise outputs reuse the landing buffers (dead after the casts)
        x2v = [xb[0].rearrange("p (r w) -> p r w", w=W),
               xb[1].rearrange("p (r w) -> p r w", w=W)]

        # ---- small tiles ----
        def sm(shape, nm, dt=f32):
            return small.tile(shape, dt, name=nm)
        xsum = [sm([P, NLOAD], f"xsum{s}") for s in range(2)]
        x1s = [sm([P, 16], f"x1s{s}") for s in range(2)]
        x2s = [sm([P, 24], f"x2s{s}") for s in range(2)]
        xsumT = [sm([P, 1], f"xsumT{s}") for s in range(2)]
        x1sT = [sm([P, 1], f"x1sT{s}") for s in range(2)]
        x2sT = [sm([P, 1], f"x2sT{s}") for s in range(2)]
        r1s = [sm([E, 1], f"r1s{s}") for s in range(2)]
        r2s = [sm([E, 1], f"r2s{s}") for s in range(2)]
        r1m = [sm([E, E], f"r1m{s}") for s in range(2)]
        r2m = [sm([E, E], f"r2m{s}") for s in range(2)]
        r1b = [sm([P, E], f"r1b{s}") for s in range(2)]
        r2b = [sm([P, E], f"r2b{s}") for s in range(2)]
        k1c = [sm([P, 64], f"k1c{s}") for s in range(2)]
        k2c = [sm([P, 9], f"k2c{s}") for s in range(2)]
        mm1w = [sm([P, P], f"mm1w{s}", f32r) for s in range(2)]
        mm1sw = [sm([P, P], f"mm1sw{s}", f32r) for s in range(2)]
        dwt = [sm([P, 9 * P], f"dwt{s}", f32r) for s in range(2)]
        seh = [sm([SE_HID, 1], f"seh{s}") for s in range(2)]
        s1c = [sm([P, 1], f"s1c{s}") for s in range(2)]
        s2c = [sm([P, 1], f"s2c{s}") for s in range(2)]
        ones4 = sm([E, P], "ones4")
        warm = sm([1, 1], "warm")

        # ---- ACT table warm + structural zeroing (all idle time).
        # f32r tiles cannot be memset; zero them with x0.0 compute writes
        # from an already-loaded const region (waits the early-const DMA).
        nc.scalar.activation(warm[:], cblob[0:1, 0:1], SIGM, bias=0.0, scale=1.0)
        nc.gpsimd.memset(ones4[:], 1.0)
        zsrc = cblob[:, 0:PADW]
        for s in range(BLOC):
            nc.vector.tensor_scalar_mul(mm1w[s][:], cblob[:, 0:P], 0.0)
            nc.vector.tensor_scalar_mul(mm1sw[s][:], cblob[:, 0:P], 0.0)
            nc.vector.tensor_scalar_mul(x1v[s][:, 0, :], zsrc[:, 0:PADW], 0.0)
            nc.vector.tensor_scalar_mul(x1v[s][:, PADH - 1, :], zsrc[:, 0:PADW], 0.0)
            nc.vector.tensor_scalar_mul(x1v[s][:, :, 0], zsrc[:, 0:PADH], 0.0)
            nc.vector.tensor_scalar_mul(x1v[s][:, :, PADW - 1], zsrc[:, 0:PADH], 0.0)

        # ---- helpers ----
        dum_ps = psum.tile([P, DUMCOL], f32, tag="dum", bufs=1)
        dum_src = small.tile([P, DUMCOL], f32r, name="dum_src")
        dgate = small.tile([P, DUMCOL], f32r, name="dgate")
        nc.vector.tensor_scalar_mul(dum_src[:], cblob[:, 0:DUMCOL], 0.0)
        nc.vector.tensor_scalar_mul(dgate[:], cblob[:, 0:DUMCOL], 0.0)
        dum_lhs = dum_src[:, 0:P]

        def dummies(n):
            # ungated: ready as soon as dum_src exists
            for _ in range(n):
                nc.tensor.matmul(dum_ps[:], dum_lhs, dum_src[:, 0:DUMCOL],
                                 start=True, stop=True)

        def dgate_mark():
            # rewrite the gate tile on DVE at this point in its stream; the
            # next gated-dummy batch becomes ready only once this runs
            nc.vector.tensor_scalar_mul(dgate[:], cblob[:, 0:DUMCOL], 0.0)

        def gdummies(n):
            # gated: wait the latest dgate version, so the scheduler cannot
            # hoist these bridge dummies ahead of their phase
            for _ in range(n):
                nc.tensor.matmul(dum_ps[:], dum_lhs, dgate[:, 0:DUMCOL],
                                 start=True, stop=True)

        def xsum_partial(s, i, eng):
            # fp32 -> fp32r rounding cast + spatial-sum side channel
            src = xb[s][:, LP * i:LP * (i + 1)]
            dst = xr[s][:, LP * i:LP * (i + 1)]
            if eng == 'A':
                nc.scalar.activation(dst, src, COPY, bias=0.0, scale=1.0,
                                     accum_out=xsum[s][:, i:i + 1])
            else:
                nc.vector.tensor_scalar(out=dst, in0=src, scalar1=1.0,
                                        scalar2=None, op0=MULT, op1=ADD,
                                        accum_out=xsum[s][:, i:i + 1])

        def r1_mm_pre(s):
            ps = psum.tile([E, 1], f32, tag="sm", bufs=1, name="smps")
            nc.tensor.matmul(ps[:], ct["rw1"], xsumT[s][:], start=True, stop=True)
            return ps

        def r_chain_smalls(s, pre_ps, rs, rm, rb_, kc, wsrc, ncols, rbias):
            # ACT sigmoid; DVE mask-mul; PE bcast matmul; DVE copy + mix
            nc.scalar.activation(rs[:], pre_ps[:], SIGM, bias=rbias, scale=1.0)
            nc.vector.tensor_scalar_mul(rm[:], ct["maskE"], rs[:, 0:1])
            bp = psum.tile([P, E], f32, tag="sm", bufs=1, name="smps")
            nc.tensor.matmul(bp[:], ones4[:], rm[:], start=True, stop=True)
            nc.vector.tensor_copy(rb_[:], bp[:])
            nc.vector.tensor_scalar_mul(kc[:], wsrc[:, 0:ncols], rb_[:, 0:1])
            for e in range(1, E):
                nc.vector.scalar_tensor_tensor(
                    kc[:], wsrc[:, e * ncols:(e + 1) * ncols],
                    rb_[:, e:e + 1], kc[:], op0=MULT, op1=ADD)

        def build_mm1(s):
            nc.vector.tensor_scalar_mul(mm1w[s][0:64, 0:64], k1c[s][0:64, :], 1.0)
            nc.vector.tensor_scalar_mul(mm1w[s][64:128, 64:128], k1c[s][64:128, :], 1.0)
            nc.vector.tensor_scalar_mul(mm1sw[s][0:64, 64:128], k1c[s][0:64, :], 1.0)
            nc.vector.tensor_scalar_mul(mm1sw[s][64:128, 0:64], k1c[s][64:128, :], 1.0)

        def m1_chunk(s, g):
            ps = psum.tile([P, 448], f32, tag="mm", bufs=6, name="mmps")
            nc.tensor.matmul(ps[:], mm1w[s][:],
                             xrv[s][:, 4 * g:4 * g + 4, :],
                             start=True, stop=True)
            return ps

        def m1_evac(s, g, ps, eng):
            dst = x1v[s][:, 1 + 4 * g:1 + 4 * g + 4, 1:1 + W]
            src = ps[:, 0:448].rearrange("p (r w) -> p r w", w=W)
            acc = x1s[s][:, g:g + 1]
            if eng == 'A':
                nc.scalar.activation(dst, src, RELU, bias=ct["bn1b"], scale=1.0,
                                     accum_out=acc)
            else:
                # two ops: bias+relu, then in-place copy carrying accum_out
                # (DVE two-scalar ts with accum_out mis-executes on HW)
                nc.vector.tensor_scalar(out=dst, in0=src, scalar1=ct["bn1b"],
                                        scalar2=0.0, op0=ADD, op1=MAX)
                nc.vector.tensor_scalar(out=dst, in0=dst.bitcast(f32),
                                        scalar1=1.0, scalar2=None,
                                        op0=MULT, op1=ADD, accum_out=acc)

        def m1_halo(s):
            # swap-block conv of half-boundary rows -> pad halo rows
            h0 = psum.tile([P, 112], f32, tag="mm", bufs=6, name="mmps")
            nc.tensor.matmul(h0[:], mm1sw[s][:], xrv[s][:, 0, :],
                             start=True, stop=True)
            nc.scalar.activation(x1v[s][0:64, PADH - 1, 1:1 + W], h0[0:64, :],
                                 RELU, bias=ct["bn1b"][0:64, :], scale=1.0)
            h1 = psum.tile([P, 112], f32, tag="mm", bufs=6, name="mmps")
            nc.tensor.matmul(h1[:], mm1sw[s][:], xrv[s][:, HH - 1, :],
                             start=True, stop=True)
            nc.scalar.activation(x1v[s][64:128, 0, 1:1 + W], h1[64:128, :],
                                 RELU, bias=ct["bn1b"][64:128, :], scale=1.0)

        def build_dwt(s):
            for t in range(9):
                nc.vector.tensor_scalar_mul(dwt[s][:, t * P:(t + 1) * P],
                                            ct["i128"], k2c[s][:, t:t + 1])

        def dw_pe_group(s, gi):
            c0, nch = PE_DW_GROUPS[gi]
            pss = []
            rows = []
            for ci in range(nch):
                nr = PE_DW[c0 + ci]
                r0 = sum(PE_DW[:c0 + ci])
                pss.append(psum.tile([P, 448], f32, tag="mm", bufs=6, name="mmps"))
                rows.append((r0, nr))
            for t in range(9):
                dy, dx = divmod(t, 3)
                for ci in range(nch):
                    r0, nr = rows[ci]
                    nc.tensor.matmul(
                        pss[ci][:, 0:nr * W],
                        dwt[s][:, t * P:(t + 1) * P],
                        x1v[s][:, r0 + dy:r0 + dy + nr, dx:dx + W],
                        start=(t == 0), stop=(t == 8))
            return pss, rows

        def dw_pe_evac(s, pss, rows, cols, eng='A'):
            for ci, (ps, (r0, nr)) in enumerate(zip(pss, rows)):
                dst = x2v[s][:, r0:r0 + nr, :]
                srcv = ps[:, 0:nr * W].rearrange("p (r w) -> p r w", w=W)
                acc = x2s[s][:, cols + ci:cols + ci + 1]
                if eng == 'A':
                    nc.scalar.activation(dst, srcv, RELU, bias=ct["bn2b"],
                                         scale=1.0, accum_out=acc)
                else:
                    nc.vector.tensor_scalar(out=dst, in0=srcv,
                                            scalar1=ct["bn2b"], scalar2=0.0,
                                            op0=ADD, op1=MAX)
                    nc.vector.tensor_scalar(out=dst, in0=dst, scalar1=1.0,
                                            scalar2=None, op0=MULT, op1=ADD,
                                            accum_out=acc)

        def dw_vec_chunk(s, r0, nr, eng):
            acc = stage.tile([P, 448], f32, tag="acc" + eng, bufs=3, name="acc")
            accv = acc[:, 0:nr * W].rearrange("p (r w) -> p r w", w=W)
            e = nc.vector if eng == 'D' else nc.gpsimd
            e.tensor_scalar(out=accv, in0=x1v[s][:, r0:r0 + nr, 0:W].bitcast(f32),
                            scalar1=k2c[s][:, 0:1], scalar2=ct["bn2b"],
                            op0=MULT, op1=ADD)
            for t in range(1, 9):
                dy, dx = divmod(t, 3)
                e.scalar_tensor_tensor(
                    accv, x1v[s][:, r0 + dy:r0 + dy + nr, dx:dx + W].bitcast(f32),
                    k2c[s][:, t:t + 1], accv, op0=MULT, op1=ADD)
            return acc

        def dw_pool_chunk(s, r0, nr):
            # Pool lacks scalar_tensor_tensor: tap product via tensor_scalar
            # into a temp, accumulate via tensor_tensor add.
            acc = stage.tile([P, 448], f32, tag="accP", bufs=2, name="acc")
            tmp = stage.tile([P, 448], f32, tag="tmpP", bufs=2, name="tmp")
            accv = acc[:, 0:nr * W].rearrange("p (r w) -> p r w", w=W)
            tmpv = tmp[:, 0:nr * W].rearrange("p (r w) -> p r w", w=W)
            nc.gpsimd.tensor_scalar(out=accv,
                                    in0=x1v[s][:, r0:r0 + nr, 0:W].bitcast(f32),
                                    scalar1=k2c[s][:, 0:1], scalar2=ct["bn2b"],
                                    op0=MULT, op1=ADD)
            for t in range(1, 9):
                dy, dx = divmod(t, 3)
                nc.gpsimd.tensor_scalar_mul(
                    tmpv, x1v[s][:, r0 + dy:r0 + dy + nr, dx:dx + W].bitcast(f32),
                    k2c[s][:, t:t + 1])
                nc.gpsimd.tensor_tensor(out=accv, in0=accv, in1=tmpv, op=ADD)
            return acc

        def dw_vec_evac(s, r0, nr, acc, col, eng):
            # relu into acc (bias folded into tap 0), then copy acc -> x2
            # carrying the accum_out side-channel (baseline-proven shapes)
            e = nc.vector if eng == 'D' else nc.gpsimd
            accv = acc[:, 0:nr * W].rearrange("p (r w) -> p r w", w=W)
            e.tensor_scalar(out=accv, in0=accv, scalar1=0.0, scalar2=0.0,
                            op0=MAX, op1=ADD)
            e.tensor_scalar(out=x2v[s][:, r0:r0 + nr, :], in0=accv,
                            scalar1=1.0, scalar2=None, op0=MULT, op1=ADD,
                            accum_out=x2s[s][:, col:col + 1])

        def se_mms(s):
            se1 = psum.tile([SE_HID, 1], f32, tag="sm", bufs=1, name="smps")
            nc.tensor.matmul(se1[:], ct["sew1a"], x1sT[s][:], start=True, stop=False)
            nc.tensor.matmul(se1[:], ct["sew1b"], x2sT[s][:], start=False, stop=True)
            nc.scalar.activation(seh[s][:], se1[:], RELU, bias=ct["seb1"], scale=1.0)
            g1 = psum.tile([P, 1], f32, tag="sm", bufs=1, name="smps")
            nc.tensor.matmul(g1[:], ct["sew2a"], seh[s][:], start=True, stop=True)
            nc.scalar.activation(s1c[s][:], g1[:], SIGM, bias=ct["seb2a"], scale=1.0)
            g2 = psum.tile([P, 1], f32, tag="sm", bufs=1, name="smps")
            nc.tensor.matmul(g2[:], ct["sew2b"], seh[s][:], start=True, stop=True)
            nc.scalar.activation(s2c[s][:], g2[:], SIGM, bias=ct["seb2b"], scale=1.0)

        GR = 14                  # gate/store piece rows

        def gate_piece(s, half_src, pi, eng):
            # half_src: 0 -> x1 (from x1pad interior), 1 -> x2
            r0 = GR * pi
            if half_src == 0:
                src = x1v[s][:, 1 + r0:1 + r0 + GR, 1:1 + W].bitcast(f32)
                sc = s1c[s][:, 0:1]
            else:
                src = x2v[s][:, r0:r0 + GR, :]
                sc = s2c[s][:, 0:1]
            nbuf = {'D': 2, 'A': 1, 'P': 2}[eng]
            gst = stage.tile([P, GR * W], f32, tag="gst" + eng, bufs=nbuf,
                             name="gst")
            gv = gst.rearrange("p (r w) -> p r w", w=W)
            if eng == 'D':
                nc.vector.tensor_scalar_mul(gv, src, sc)
            elif eng == 'P':
                nc.gpsimd.tensor_scalar_mul(gv, src, sc)
            else:
                nc.scalar.activation(gv, src, COPY, bias=0.0, scale=sc)
            return gst

        y_ap1 = y_d.ap()[:, 0:64].rearrange("b c r w -> b c (r w)")
        y_ap2 = y_d.ap()[:, 64:128].rearrange("b c r w -> b c (r w)")

        def store_piece(s, half_src, pi, gst):
            dst = (y_ap1 if half_src == 0 else y_ap2)
            for hf in range(2):
                off = HH * W * hf + GR * W * pi
                nc.sync.dma_start(dst[s, :, off:off + GR * W],
                                  gst[64 * hf:64 * hf + 64, :])

        # ================= emission schedule =================
        # -- load0 window --
        dummies(DUM_A)
        xsum_partial(0, 0, 'A')
        xsum_partial(0, 1, 'A')
        xsum_partial(0, 2, 'A')
        xsum_partial(0, 3, 'D')
        nc.vector.reduce_sum(out=xsumT[0][:], in_=xsum[0][:, 0:NLOAD], axis=AX)
        pre0 = r1_mm_pre(0)
        r_chain_smalls(0, pre0, r1s[0], r1m[0], r1b[0], k1c[0], ct["w1T"], 64,
                       ct["rb1"])
        build_mm1(0)
        dgate_mark()
        gdummies(DUM_B)

        # -- M1_0 --
        for g in range(14):
            ps = m1_chunk(0, g)
            m1_evac(0, g, ps, M1_EVAC_0[g])
        m1_halo(0)
        nc.vector.reduce_sum(out=x1sT[0][:], in_=x1s[0][:, 0:14], axis=AX)
        dgate_mark()
        gdummies(DUM_C)

        # -- r2_0 --
        pre = psum.tile([E, 1], f32, tag="sm", bufs=1, name="smps")
        nc.tensor.matmul(pre[:], ct["rw2"], x1sT[0][:], start=True, stop=True)
        r_chain_smalls(0, pre, r2s[0], r2m[0], r2b[0], k2c[0], ct["w2"], 9,
                       ct["rb2"])
        build_dwt(0)
        dgate_mark()
        gdummies(DUM_D)

        # -- DW_0 + sample-1 prep interleave --
        xsum_partial(1, 0, 'A')
        xsum_partial(1, 1, 'A')
        xsum_partial(1, 2, 'D')
        xsum_partial(1, 3, 'D')
        nc.vector.reduce_sum(out=xsumT[1][:], in_=xsum[1][:, 0:NLOAD], axis=AX)
        pss1, rows1 = dw_pe_group(0, 0)  # PE G1 (4 chunks)
        dw_pe_evac(0, pss1, rows1, 0)

        # r1_1 chain (PE mms slot in mid-DW stream)
        pre1 = r1_mm_pre(1)
        r_chain_smalls(1, pre1, r1s[1], r1m[1], r1b[1], k1c[1], ct["w1T"], 64,
                       ct["rb1"])
        build_mm1(1)

        # dve DW chunks 0-1
        r0, nr = DVE_DW_ROWS[0]
        acc = dw_vec_chunk(0, r0, nr, 'D')
        dw_vec_evac(0, r0, nr, acc, 11, 'D')

        pss2, rows2 = dw_pe_group(0, 1)  # PE G2 (3 chunks)
        dw_pe_evac(0, pss2, rows2, 4)
        r0, nr = DVE_DW_ROWS[1]
        acc = dw_vec_chunk(0, r0, nr, 'D')
        dw_vec_evac(0, r0, nr, acc, 12, 'D')
        pss3, rows3 = dw_pe_group(0, 2)  # PE G3
        dw_pe_evac(0, pss3, rows3, 7)
        r0, nr = DVE_DW_ROWS[2]
        acc = dw_vec_chunk(0, r0, nr, 'D')
        dw_vec_evac(0, r0, nr, acc, 13, 'D')
        pss4, rows4 = dw_pe_group(0, 3)  # PE G4
        dw_pe_evac(0, pss4, rows4, 14)

        nc.vector.reduce_sum(out=x2sT[0][:], in_=x2s[0][:, 0:16], axis=AX)
        dgate_mark()
        gdummies(DUM_SE0)
        se_mms(0)

        # -- gate_0 pieces + M1_1 --
        GATE_SEQ = [(0, 0, 'D'), (0, 1, 'P'), (0, 2, 'D'), (0, 3, 'P'),
                    (1, 0, 'D'), (1, 1, 'P'), (1, 2, 'D'), (1, 3, 'P')]
        gp = gate_piece(0, *GATE_SEQ[0])
        store_piece(0, GATE_SEQ[0][0], GATE_SEQ[0][1], gp)
        gp = gate_piece(0, *GATE_SEQ[1])
        store_piece(0, GATE_SEQ[1][0], GATE_SEQ[1][1], gp)
        for g in range(14):
            ps = m1_chunk(1, g)
            m1_evac(1, g, ps, M1_EVAC_1[g])
        m1_halo(1)
        for hs, pi, eng in GATE_SEQ[2:]:
            gp = gate_piece(0, hs, pi, eng)
            store_piece(0, hs, pi, gp)
        nc.vector.reduce_sum(out=x1sT[1][:], in_=x1s[1][:, 0:14], axis=AX)
        dgate_mark()
        gdummies(DUM_E)

        # -- r2_1 --
        pre = psum.tile([E, 1], f32, tag="sm", bufs=1, name="smps")
        nc.tensor.matmul(pre[:], ct["rw2"], x1sT[1][:], start=True, stop=True)
        r_chain_smalls(1, pre, r2s[1], r2m[1], r2b[1], k2c[1], ct["w2"], 9,
                       ct["rb2"])
        build_dwt(1)
        dgate_mark()
        gdummies(DUM_F)

        # -- DW_1 --
        pss1, rows1 = dw_pe_group(1, 0)
        dw_pe_evac(1, pss1, rows1, 0)
        r0, nr = DVE_DW_ROWS[0]
        acc = dw_vec_chunk(1, r0, nr, 'D')
        dw_vec_evac(1, r0, nr, acc, 11, 'D')
        pss2, rows2 = dw_pe_group(1, 1)
        dw_pe_evac(1, pss2, rows2, 4)
        r0, nr = DVE_DW_ROWS[1]
        acc = dw_vec_chunk(1, r0, nr, 'D')
        dw_vec_evac(1, r0, nr, acc, 12, 'D')
        pss3, rows3 = dw_pe_group(1, 2)
        dw_pe_evac(1, pss3, rows3, 7)
        r0, nr = DVE_DW_ROWS[2]
        acc = dw_vec_chunk(1, r0, nr, 'D')
        dw_vec_evac(1, r0, nr, acc, 13, 'D')
        pss4, rows4 = dw_pe_group(1, 3)
        dw_pe_evac(1, pss4, rows4, 14)

        nc.vector.reduce_sum(out=x2sT[1][:], in_=x2s[1][:, 0:16], axis=AX)
        dgate_mark()
        gdummies(DUM_SE1)
        se_mms(1)

        # -- gate_1 + store_1 --
        for hs, pi, eng in GATE_SEQ:
            gp = gate_piece(1, hs, pi, eng)
            store_piece(1, hs, pi, gp)


# ---------------- build + run ----------------
_CACHE = {}


def _build():
    if "nc" in _CACHE:
        return _CACHE["nc"]
    nc = bacc.Bacc("TRN2", target_bir_lowering=False, debug=False,
                   enable_asserts=False, num_devices=NCORES)
    x_d = nc.dram_tensor("x_in", [BLOC, C_IN, H, W], f32, kind="ExternalInput")
    y_d = nc.dram_tensor("y_out", [BLOC, 2 * INIT, H, W], f32,
                         kind="ExternalOutput")
    cblob_d = nc.dram_tensor("cblob", [P, CBW], f32, kind="ExternalInput")
    with tile.TileContext(nc) as tc:
        _emit(tc, x_d, y_d, cblob_d)
    nc.compile()
    _CACHE["nc"] = nc
    return nc


def _run(inputs, trace=False):
    nc = _build()
    blob = _pack_consts({k: v for k, v in inputs.items() if k != "x"})
    x = np.ascontiguousarray(np.asarray(inputs["x"], dtype=np.float32))
    in_maps = []
    for ci in range(NCORES):
        in_maps.append({"x_in": np.ascontiguousarray(x[BLOC * ci:BLOC * (ci + 1)]),
                        "cblob": blob})
    res = run_bass_kernel_spmd(nc, in_maps, list(range(NCORES)), trace=trace)
    out = np.concatenate([res.results[ci]["y_out"] for ci in range(NCORES)], axis=0)
    return out, res


def kernel(**inputs):
    out, _ = _run(inputs)
    return out
